# revision 32
# baseline (speedup 1.0000x reference)
"""Contextual-attention kernel for Trainium2, batch-parallel over 8 NeuronCores.

Per core (one image, feature [256,64,64], shared mask [128,128]):
  1. fd = nearest-downsampled feature, zero-padded       [256, 34, 34]
  2. RW deconv patch banks prebuilt early (f-dependent only): PE transposes
     of contiguous-staged (u,v) grids of f_pad2 -> RW[cc][q, c]; PSUM->SBUF
     copies batched 4 transposes at a time, split across DVE/Act.
  3. Gram scores S[q,p] = sum over 9 patch-shift outer products (PE matmuls;
     lhsT from contiguous q-strip staging), scaled by
     rnorm[q] = 1/max(||patch_q||, eps)
  4. fuse conv 1 (diag +-1, row-major) and fuse conv 2 (diag +-1, col-major
     incl. wrap slivers): implemented as PE identity-shift matmuls
     accumulating in PSUM; chunk-boundary single rows via gpsimd
     accumulate-DMAs.  Mask (mm_q) folded into the fuse2 PSUM->SBUF copies.
  5. per-p max via PE transposes (f32r) + free-dim reduce; max row built by
     PE transposes (not DMA); broadcast via 1xK ones matmul; subtract,
     exp(10*x) on ScalarE -> bf16
  6. denominators via ones matmul over q, reciprocal; final weights into
     zero-padded A_pad [q, 34, 34]
  7. deconv: 512 accumulating matmuls vs prebuilt RW -> out[c, parity
     grids]; *0.25; stores split across SP/Act DMA queues.

SBUF: one slot-shared "work" pool (4 x 32KB slots, tag "wk") serves all
large buffers with disjoint lifetimes.
"""
import sys

sys.path.insert(0, "/opt/trn_rl_repo")

import numpy as np

import concourse.bass as bass
import concourse.bacc as bacc_mod
import concourse.mybir as mybir
import concourse.tile as tile
from concourse.masks import make_identity
from concourse.bass_utils import run_bass_kernel_spmd

F32 = mybir.dt.float32
F32R = mybir.dt.float32r
BF16 = mybir.dt.bfloat16
AX = mybir.AxisListType
OP = mybir.AluOpType
ACT = mybir.ActivationFunctionType

N_CORES = 8
C, H, W = 256, 64, 64
SCALE = 10.0
EPS = 1e-4


def build_nc(gram_dt="f32r", dec_dt="bf16", reps=1):
    nc = bacc_mod.Bacc("TRN2", target_bir_lowering=False, debug=False)
    feat = nc.dram_tensor("feature", [C, H, W], F32, kind="ExternalInput")
    mask0 = nc.dram_tensor("mask0", [128, 128], F32, kind="ExternalInput")
    out_d = nc.dram_tensor("out", [C, H, W], F32, kind="ExternalOutput")

    assert dec_dt in ("bf16", "f32")
    ddt = BF16 if dec_dt == "bf16" else F32
    gdt = F32R if gram_dt == "f32r" else F32

    with tile.TileContext(nc) as tc:
        with (
            tc.tile_pool(name="fpool", bufs=1) as fpl,
            tc.tile_pool(name="work", bufs=4) as wk,
            tc.tile_pool(name="wr", bufs=4) as wr,
            tc.tile_pool(name="qbp", bufs=2) as qbp,
            tc.tile_pool(name="sml", bufs=1) as sml,
            tc.tile_pool(name="acc", bufs=4, space="PSUM") as pacc,
            tc.tile_pool(name="ptp", bufs=3, space="PSUM") as ptp,
        ):
            for rep in range(reps):
                _body(nc, tc, fpl, wk, wr, qbp, sml, pacc, ptp,
                      feat, mask0, out_d, gdt, ddt, rep)
    nc.finalize()
    return nc


def _body(nc, tc, fpl, wk, wr, qbp, sml, pacc, ptp, feat, mask0, out_d, gdt, ddt, rep):
    # ---------------- constants ----------------
    ident = sml.tile([128, 128], F32, tag="ident")
    make_identity(nc, ident)
    identr_t = sml.tile([128, 128], F32R, tag="identr")
    nc.vector.tensor_copy(identr_t[:], ident[:])
    identr = identr_t[:]
    ones128 = sml.tile([128, 128], F32, tag="ones128")
    nc.any.memset(ones128[:], 1.0)
    ones_bf = sml.tile([128, 128], BF16, tag="ones_bf")
    nc.any.memset(ones_bf[:], 1.0)
    ident_bq = sml.tile([128, 128], BF16, tag="ident_bf")
    nc.vector.tensor_copy(ident_bq[:], ident[:])
    ident_d = ident_bq if ddt == BF16 else ident
    ones1 = sml.tile([1, 128], F32, tag="ones1")
    nc.any.memset(ones1[:], 1.0)

    # shift matrices for the fuse stages: SH(s)[k, m] = 1 iff k == m + s,
    # so matmul(out, SH(s), rhs) gives out[m] = rhs[m+s] (zero off-range).
    shtmp = sml.tile([128, 128], F32, tag="shtmp")

    def make_shift(name, s, zero_cols=()):
        nc.gpsimd.memset(shtmp[:], 0.0)
        nc.gpsimd.affine_select(
            out=shtmp[:], in_=shtmp[:], compare_op=OP.not_equal, fill=1.0,
            base=-s, pattern=[[-1, 128]], channel_multiplier=1)
        for c in zero_cols:
            nc.gpsimd.memset(shtmp[:, c:c + 1], 0.0)
        t = sml.tile([128, 128], F32R, tag=name)
        nc.vector.tensor_copy(t[:], shtmp[:])
        return t[:]

    sh1p = make_shift("sh1p", 1)
    sh1m = make_shift("sh1m", -1)
    sh32p = make_shift("sh32p", 32)
    sh32m = make_shift("sh32m", -32)
    sh96m = make_shift("sh96m", -96)
    sh96p = make_shift("sh96p", 96)
    # chunk-7 qy=31 wrap: out[m] = rhs[m-95] only for m in 96..126
    w7 = make_shift("w7", -95, zero_cols=(95, 127))
    # chunk-0 qy=0 wrap: out[m] = rhs[m+95] only for m in 1..31
    w0 = make_shift("w0", 95, zero_cols=(0, 32))
    # fuse1 cross-chunk stitch rows: single-element selectors
    st_p = make_shift("st_p", -127)   # out[127] = rhs[0]
    st_m = make_shift("st_m", 127)    # out[0] = rhs[127]

    # ---------------- stage 0: loads & padded layouts ----------------
    # contiguous feature loads on parallel DMA queues (SP cc=0, Act cc=1)
    fraw, fdp = [], []
    for cc in range(2):
        t = wk.tile([128, 64, 64], F32, tag="wk", name=f"fraw_{rep}_{cc}")
        eng = nc.sync if cc == 0 else nc.scalar
        eng.dma_start(t[:], feat[cc * 128:(cc + 1) * 128])
        fraw.append(t)
    zbf = qbp.tile([128, 1156], F32, tag="qb", name=f"zbf_{rep}")
    nc.gpsimd.memset(zbf[:], 0.0)
    for cc in range(2):
        t = fpl.tile([128, 34, 34], gdt, tag=f"fdp_{cc}")
        nc.vector.tensor_copy(t[:].rearrange("p a b -> p (a b)"), zbf[:])
        nc.vector.tensor_copy(t[:, 1:33, 1:33], fraw[cc][:, 0:64:2, 0:64:2])
        fdp.append(t)

    # ---------------- stage 0b: deconv RW banks (built in pieces) --------
    # RW[cc][q-part, qc, u*4+v, c] = f_pad2[c, 2qy+u, 2qx+v] transposed.
    # (u,v) grouped in 4s so one PSUM->SBUF copy covers 4 transposes.
    # Groups are emitted interleaved with the gram/fuse phases to fill
    # PE dependency stalls and spread the DVE/Act copy load.
    RW = [wk.tile([128, 8, 16, 128], ddt, tag="wk", name=f"rw_{rep}_{cc}")
          for cc in range(2)]

    def rw_group(cc, grp):
        rw = RW[cc]
        gbs = []
        for k in range(4):
            uv = grp * 4 + k
            u, v = uv // 4, uv % 4
            gb = wr.tile([128, 1024], ddt, tag="w", name=f"gb_{rep}_{cc}_{uv}")
            gv = gb[:].rearrange("p (a b) -> p a b", a=32)
            # grid rows r(qy) = 2qy+u-1; u=0 -> qy=0 OOB, u=3 -> qy=31 OOB
            y0, y1 = (1, 32) if u == 0 else ((0, 31) if u == 3 else (0, 32))
            x0, x1 = (1, 32) if v == 0 else ((0, 31) if v == 3 else (0, 32))
            if u == 0:
                nc.gpsimd.memset(gv[:, 0, :], 0.0)
            elif u == 3:
                nc.gpsimd.memset(gv[:, 31, :], 0.0)
            if v == 0:
                nc.gpsimd.memset(gv[:, y0:y1, 0], 0.0)
            elif v == 3:
                nc.gpsimd.memset(gv[:, y0:y1, 31], 0.0)
            r0, c0 = 2 * y0 + u - 1, 2 * x0 + v - 1
            r1 = min(r0 + 2 * (y1 - y0), 64)
            c1 = min(c0 + 2 * (x1 - x0), 64)
            nc.scalar.copy(gv[:, y0:y1, x0:x1], fraw[cc][:, r0:r1:2, c0:c1:2])
            gbs.append(gb)
        for qc in range(8):
            ps = ptp.tile([128, 512], ddt, tag="tp")
            for k in range(4):
                nc.tensor.transpose(ps[:, 128 * k:128 * (k + 1)],
                                    gbs[k][:, 128 * qc: 128 * (qc + 1)],
                                    ident_d[:])
            dst = rw[:, qc, 4 * grp: 4 * (grp + 1), :].rearrange(
                "p a b -> p (a b)")
            if qc % 2 == 0:
                nc.vector.tensor_copy(dst, ps[:])
            else:
                nc.scalar.copy(dst, ps[:])

    rw_group(0, 0)
    rw_group(0, 1)

    # ---------------- stage 1: mask -> mm_q [128, 8] ----------------
    msc = sml.tile([1, 3204], F32, tag="msc")
    for k, (dy, dx) in enumerate(((0, 0), (0, 1), (1, 0), (1, 1))):
        off = 0 if k == 0 else 1024
        dst = msc[:, off:off + 1024].rearrange("o (a b) -> o a b", a=32)
        nc.sync.dma_start(dst, mask0[dy::4, dx::4][None])
        if k > 0:
            nc.gpsimd.tensor_add(msc[:, 0:1024], msc[:, 0:1024],
                                 msc[:, 1024:2048])
    msum = msc[:, 0:1024].rearrange("o (a b) -> o a b", a=32)
    mdp = msc[:, 2048:3204].rearrange("o (a b) -> o a b", a=34)
    mbx = msc[:, 1024:2112].rearrange("o (a b) -> o a b", a=34)
    nc.gpsimd.memset(mdp[:], 0.0)
    nc.gpsimd.tensor_scalar(mdp[:, 1:33, 1:33], msum[:], 2.5, None, OP.is_ge)
    nc.gpsimd.tensor_add(mbx[:], mdp[:, :, 0:32], mdp[:, :, 1:33])
    nc.gpsimd.tensor_add(mbx[:], mbx[:], mdp[:, :, 2:34])
    mbox = msc[:, 0:1024].rearrange("o (a b) -> o a b", a=32)
    nc.gpsimd.tensor_add(mbox[:], mbx[:, 0:32, :], mbx[:, 1:33, :])
    nc.gpsimd.tensor_add(mbox[:], mbox[:], mbx[:, 2:34, :])
    mmrow = msc[:, 2112:3136]
    nc.gpsimd.tensor_scalar(mmrow[:].rearrange("o (a b) -> o a b", a=32),
                            mbox[:], 0.0, None, OP.is_equal)
    mm_q = sml.tile([128, 8], F32, tag="mm_q")

    # ---------------- stage 1b: rnorm_q [128, 8] ----------------
    nsc = sml.tile([128, 2244], F32, tag="nsc")
    ssq = nsc[:, 0:1156].rearrange("p (a b) -> p a b", a=34)
    nbx = nsc[:, 1156:2244].rearrange("p (a b) -> p a b", a=34)
    sq = []
    for cc in range(2):
        t = qbp.tile([128, 1156], F32, tag="qb", name=f"sq_{rep}_{cc}")
        nc.scalar.square(t[:], fdp[cc][:].rearrange("p a b -> p (a b)"))
        sq.append(t)
    for (o, n) in ((0, 512), (512, 512), (1024, 132)):
        ps = pacc.tile([128, 512], F32, tag="acc")
        for cc in range(2):
            nc.tensor.matmul(ps[:, :n], ones128[:], sq[cc][:, o:o + n],
                             start=(cc == 0), stop=(cc == 1))
        nc.vector.tensor_copy(nsc[:, o:o + n], ps[:, :n])
    nc.vector.tensor_add(nbx[:], ssq[:, :, 0:32], ssq[:, :, 1:33])
    nc.vector.tensor_add(nbx[:], nbx[:], ssq[:, :, 2:34])
    n2 = nsc[:, 0:1024].rearrange("p (a b) -> p a b", a=32)
    nc.vector.tensor_add(n2[:], nbx[:, 0:32, :], nbx[:, 1:33, :])
    nc.vector.tensor_add(n2[:], n2[:], nbx[:, 2:34, :])
    nrm = nsc[:, 1156:2180]
    rnm = nsc[:, 0:1024]
    nc.scalar.sqrt(nrm[:], nsc[:, 0:1024])
    nc.vector.tensor_scalar_max(nrm[:], nrm[:], EPS)
    nc.vector.reciprocal(rnm[:], nrm[:])
    rnorm_q = sml.tile([128, 8], F32, tag="rnorm_q")
    nrm_rep = sml.tile([128, 1024], F32, tag="nrm_rep")

    # ---------------- stage 2: Gram -> M0[q, p] (symmetric) --------------
    # G is symmetric before the rnorm scaling: compute only 256-col blocks
    # (t, g) with g >= t//2; mirror the lower blocks via PE transposes,
    # re-scaled by rnorm[p-part] * ||q||-row (scalar_tensor_tensor).
    # M0/M1 carry one zero pad column on each side of every 1024-wide
    # chunk so every fuse matmul writes a full, aligned 512-wide PSUM slab.
    M0 = wk.tile([128, 8, 1026], F32, tag="wk", name=f"m0_{rep}")
    nc.vector.memset(M0[:, :, 0:1026:1025], 0.0)
    shifts = [(i, j) for i in range(3) for j in range(3)]
    psq = ptp.tile([128, 16], F32, tag="tp", name=f"psq_{rep}")

    def gram_matmuls(t, qb):
        pss = []
        for g in range(t // 2, 4):
            ps = pacc.tile([128, 256], F32, tag="acc")
            k = 0
            for cc in range(2):
                for s, (i, j) in enumerate(shifts):
                    lhsT = qb[:, cc, s, :]
                    rhs = fdp[cc][:, i + 8 * g: i + 8 * g + 8, j:j + 32]
                    nc.tensor.matmul(ps[:], lhsT, rhs,
                                     start=(k == 0), stop=(k == 17))
                    k += 1
            pss.append((g, ps))
        return pss

    def gram_stage(t):
        qb = qbp.tile([128, 2, 9, 128], gdt, tag="qb", name=f"qb_{rep}_{t}")
        for cc in range(2):
            for s, (i, j) in enumerate(shifts):
                nc.vector.tensor_copy(
                    qb[:, cc, s, :].rearrange("p (a b) -> p a b", a=4),
                    fdp[cc][:, i + 4 * t: i + 4 * t + 4, j:j + 32])
        return gram_matmuls(t, qb)

    def gram_finish(t, pss):
        for g, ps in pss:
            nc.vector.tensor_scalar_mul(
                M0[:, t, 1 + 256 * g: 1 + 256 * (g + 1)].bitcast(F32R),
                ps[:], rnorm_q[:, t:t + 1])
        # mirror lower blocks of row t: (t, g) for g < t//2 from rows 2g/2g+1
        for g in range(t // 2):
            psT = ptp.tile([128, 256], F32, tag="tp", name=f"mir_{rep}_{t}_{g}")
            for a in range(2):
                nc.tensor.transpose(
                    psT[:, 128 * a:128 * (a + 1)].bitcast(F32R),
                    M0[:, 2 * g + a, 1 + 128 * t: 1 + 128 * t + 128]
                    .bitcast(F32R),
                    identr)
            nc.vector.scalar_tensor_tensor(
                out=M0[:, t, 1 + 256 * g: 1 + 256 * (g + 1)].bitcast(F32R),
                in0=psT[:], scalar=rnorm_q[:, t:t + 1],
                in1=nrm_rep[:, 256 * g: 256 * (g + 1)],
                op0=OP.mult, op1=OP.mult)

    # t=0 matmuls run on PE while the DVE norm chain drains; the rnorm
    # transposes come after them so the in-order PE queue is never blocked
    pss0 = gram_stage(0)
    for c8 in range(8):
        nc.tensor.transpose(psq[:, 8 + c8:9 + c8],
                            rnm[0:1, 128 * c8:128 * (c8 + 1)], ident[0:1, 0:1])
    nc.vector.tensor_copy(rnorm_q[:], psq[:, 8:16])
    gram_finish(0, pss0)
    for h in range(2):
        psn = pacc.tile([128, 512], F32, tag="acc", name=f"nr_{rep}_{h}")
        nc.tensor.matmul(psn[:], ones1[:], nrm[0:1, 512 * h:512 * (h + 1)],
                         start=True, stop=True)
        nc.vector.tensor_copy(nrm_rep[:, 512 * h:512 * (h + 1)], psn[:])
    for t in range(1, 8):
        pss = gram_stage(t)
        gram_finish(t, pss)
        if t == 1:
            rw_group(0, 2)
        elif t == 3:
            rw_group(0, 3)
        elif t == 2:
            # mask column transport (mask chain surely drained by now)
            for c8 in range(8):
                nc.tensor.transpose(psq[:, c8:c8 + 1],
                                    mmrow[0:1, 128 * c8:128 * (c8 + 1)],
                                    ident[0:1, 0:1])
            nc.vector.tensor_copy(mm_q[:], psq[:, 0:8])

    for grp in range(4):
        rw_group(1, grp)

    # ---------------- stage 3: fuse1 (diag +-1, row-major) on PE ----------
    # M1[q, j] = M0[q, j] + M0[q+1, j+1] + M0[q-1, j-1] (integer q/p index,
    # zero at bounds).  Partition shifts by identity-slice matmuls; the
    # cross-chunk single rows via gpsimd accumulate-DMAs afterwards.
    M1 = wk.tile([128, 8, 1026], F32, tag="wk", name=f"m1_{rep}")
    nc.vector.memset(M1[:, :, 0:1026:1025], 0.0)
    for ch in range(8):
        for h in range(2):
            lo = 512 * h
            ps = pacc.tile([128, 512], F32, tag="acc")
            # center (padded data col = p + 1)
            nc.tensor.matmul(ps[:], identr,
                             M0[:, ch, 1 + lo:1 + lo + 512].bitcast(F32R),
                             start=True, stop=False)
            # +1 term: out[q, p] += M0[q+1, p+1] (p=1023 hits the zero pad)
            nc.tensor.matmul(ps[:], sh1p,
                             M0[:, ch, 2 + lo:2 + lo + 512].bitcast(F32R),
                             start=False, stop=False)
            # -1 term: out[q, p] += M0[q-1, p-1] (p=0 hits the zero pad)
            nc.tensor.matmul(ps[:], sh1m,
                             M0[:, ch, lo:lo + 512].bitcast(F32R),
                             start=False, stop=False)
            # cross-chunk stitch rows via single-element selector matmuls
            if ch < 7:
                nc.tensor.matmul(ps[:], st_p,
                                 M0[:, ch + 1, 2 + lo:2 + lo + 512]
                                 .bitcast(F32R),
                                 start=False, stop=(ch == 0))
            if ch > 0:
                nc.tensor.matmul(ps[:], st_m,
                                 M0[:, ch - 1, lo:lo + 512].bitcast(F32R),
                                 start=False, stop=True)
            if h == 0:
                nc.vector.tensor_copy(
                    M1[:, ch, 1 + lo:1 + lo + 512].bitcast(F32R), ps[:])
            else:
                nc.scalar.copy(M1[:, ch, 1 + lo:1 + lo + 512].bitcast(F32R),
                               ps[:])

    # ---------------- stage 4: fuse2 (diag +-1, col-major) on PE ----------
    # col-major +1 on integer index i (grid (a, b), i = b*32 + a):
    #   q side: q+32 for qy<=30; (qy=31, qx) -> qx+1 (chunk-7 wrap)
    #   p side: p+32 for py<=30; (py=31, px) -> px+1 (free-dim sliver)
    # mask mm_q folded into the PSUM->SBUF copies.
    M2 = wk.tile([128, 8, 1024], F32, tag="wk", name=f"m2_{rep}")

    def m1p(ch_, c0, c1):
        # padded column indexing: data col p lives at padded col p + 1
        return M1[:, ch_, c0:c1].bitcast(F32R)

    for ch in range(8):
        # q-side lhsT for the +1 / -1 col-major terms:
        #   main piece within chunk, cross piece from the adjacent chunk
        qp = [(sh32p, ch), (sh96m, ch + 1) if ch < 7 else (w7, 0)]
        qm = [(sh32m, ch), (sh96p, ch - 1) if ch > 0 else (w0, 7)]
        for h in range(2):
            lo = 512 * h
            ps = pacc.tile([128, 512], F32, tag="acc")
            mm = [(ps[:], identr, m1p(ch, 1 + lo, 1 + lo + 512))]
            sl = []   # sliver matmuls -> aligned scratch psum
            if h == 0:
                # +1: out cols 0..511 (py 0..15) <- src data 32..543
                for l, c in qp:
                    mm.append((ps[:], l, m1p(c, 33, 545)))
                # -1: out cols 32..511 (py 1..15) <- src data 0..479
                for l, c in qm:
                    mm.append((ps[:, 32:512], l, m1p(c, 1, 481)))
                # -1 sliver: out p 1..31 (py=0, px 1..31) <- data 991+p
                for l, c in qm:
                    sl.append((l, m1p(c, 992, 1024)))
                sadd = (1, 32, 1, 32)   # pssl[1:32] -> ps[1:32]
            else:
                # +1 main: out cols 0..479 (py 16..30) <- src data 544..1023
                for l, c in qp:
                    mm.append((ps[:, 0:480], l, m1p(c, 545, 1025)))
                # +1 sliver: out p 992..1022 (py=31, px 0..30) <- data px+1
                for l, c in qp:
                    sl.append((l, m1p(c, 2, 34)))
                sadd = (0, 31, 480, 511)  # pssl[0:31] -> ps[480:511]
                # -1: out cols 512..1023 (py 16..31) <- src data 480..991
                for l, c in qm:
                    mm.append((ps[:], l, m1p(c, 481, 993)))
            for k, (o, l, r) in enumerate(mm):
                nc.tensor.matmul(o, l, r, start=(k == 0),
                                 stop=(k == len(mm) - 1))
            pssl = ptp.tile([128, 32], F32, tag="tp", name=f"sl_{rep}_{ch}_{h}")
            for k, (l, r) in enumerate(sl):
                nc.tensor.matmul(pssl[:], l, r, start=(k == 0),
                                 stop=(k == len(sl) - 1))
            s0, s1, d0, d1 = sadd
            slv = wr.tile([128, 32], F32, tag="w", name=f"slv_{rep}_{ch}_{h}")
            nc.vector.tensor_copy(slv[:, s0:s1], pssl[:, s0:s1])
            nc.vector.tensor_add(ps[:, d0:d1], ps[:, d0:d1], slv[:, s0:s1])
            # masked copy out (fold mm_q)
            if h == 0:
                nc.vector.tensor_scalar_mul(
                    M2[:, ch, lo:lo + 512].bitcast(F32R), ps[:],
                    mm_q[:, ch:ch + 1])
            else:
                nc.scalar.mul(M2[:, ch, lo:lo + 512].bitcast(F32R), ps[:],
                              mm_q[:, ch:ch + 1])

    # ---------------- stage 5: max, subtract, exp ----------------
    mx8 = sml.tile([128, 8, 2], F32, tag="mx8")
    for pt in range(8):
        for g in range(2):
            ps = ptp.tile([128, 512], F32, tag="tp", name=f"tpb_{rep}_{pt}_{g}")
            for t4 in range(4):
                t = 4 * g + t4
                nc.tensor.transpose(
                    ps[:, 128 * t4:128 * (t4 + 1)].bitcast(F32R),
                    M2[:, t, 128 * pt:128 * (pt + 1)].bitcast(F32R),
                    identr)
            nc.vector.reduce_max(mx8[:, pt, g:g + 1], ps[:], axis=AX.X)
    mx_all = sml.tile([128, 8], F32, tag="mx_all")
    for pt in range(8):
        nc.vector.reduce_max(mx_all[:, pt:pt + 1], mx8[:, pt, :], axis=AX.X)
    # max col [128, 8] -> row [1, 1024] via PE transposes
    mxrow = sml.tile([1, 1024], F32, tag="mxrow")
    for g in range(2):
        psr = ptp.tile([1, 512], F32, tag="tp", name=f"mxr_{rep}_{g}")
        for c4 in range(4):
            pt = 4 * g + c4
            nc.tensor.transpose(psr[0:1, 128 * c4:128 * (c4 + 1)],
                                mx_all[:, pt:pt + 1], ident[:])
        nc.vector.tensor_copy(mxrow[:, 512 * g:512 * (g + 1)], psr[:])
    E = wk.tile([128, 8, 1024], BF16, tag="wk", name=f"e_{rep}")
    psrs = []
    bcs = qbp.tile([128, 1024], F32, tag="qb", name=f"bcs_{rep}")
    for h in range(2):
        psr = pacc.tile([128, 512], F32, tag="acc", name=f"bc_{rep}_{h}")
        nc.tensor.matmul(psr[:], ones1[:], mxrow[:, 512 * h:512 * (h + 1)],
                         start=True, stop=True)
        nc.scalar.copy(bcs[:, 512 * h:512 * (h + 1)], psr[:])
        psrs.append(psr)
    # subtract/exp/denominator pipelined at (t, h) granularity; both
    # denominator PSUM groups stay open while exps stream in
    rcp = sml.tile([128, 1024], F32, tag="rcp")
    pssd = [pacc.tile([128, 512], F32, tag="acc", name=f"dn_{rep}_{h}")
            for h in range(2)]
    for t in range(8):
        for h in range(2):
            if h == 0:
                nc.vector.tensor_tensor(
                    M1[:, t, 1 + 512 * h:1 + 512 * (h + 1)].bitcast(F32R),
                    M2[:, t, 512 * h:512 * (h + 1)], psrs[h][:], OP.subtract)
            else:
                nc.gpsimd.tensor_tensor(
                    M1[:, t, 1 + 512 * h:1 + 512 * (h + 1)].bitcast(F32R),
                    M2[:, t, 512 * h:512 * (h + 1)],
                    bcs[:, 512 * h:512 * (h + 1)], OP.subtract)
            nc.scalar.activation(E[:, t, 512 * h:512 * (h + 1)],
                                 M1[:, t, 1 + 512 * h:1 + 512 * (h + 1)],
                                 ACT.Exp, bias=0.0, scale=SCALE)
            nc.tensor.matmul(pssd[h][:], ones_bf[:],
                             E[:, t, 512 * h:512 * (h + 1)],
                             start=(t == 0), stop=(t == 7))
    for h in range(2):
        nc.vector.reciprocal(rcp[:, 512 * h:512 * (h + 1)], pssd[h][:])

    # ---------------- stage 5c: final weights -> A_pad ----------------
    A_pad = wk.tile([128, 8, 34, 34], ddt, tag="wk", name=f"ap_{rep}")
    nc.gpsimd.memset(A_pad[:, :, 0:34:33, :], 0.0)
    nc.gpsimd.memset(A_pad[:, :, 1:33, 0:34:33], 0.0)
    # weight build split into row halves so the top-half deconv can start
    # while bottom-half weights are still being produced
    for (r0, r1) in ((1, 19), (19, 33)):
        for t in range(8):
            nc.vector.scalar_tensor_tensor(
                out=A_pad[:, t, r0:r1, 1:33],
                in0=E[:, t, :].rearrange("p (a b) -> p a b", a=32)
                [:, r0 - 1:r1 - 1, :],
                scalar=mm_q[:, t:t + 1],
                in1=rcp[:].rearrange("p (a b) -> p a b", a=32)
                [:, r0 - 1:r1 - 1, :],
                op0=OP.mult, op1=OP.mult)

    # ---------------- stage 6: deconv ----------------
    for cc in range(2):
        out_sb = wk.tile([128, 64, 64], F32, tag="wk", name=f"os_{rep}_{cc}")
        od = out_d[cc * 128:(cc + 1) * 128]
        for h in range(2):
            accs, cnt = {}, {}
            for ry in range(2):
                for rx in range(2):
                    accs[(ry, rx)] = pacc.tile(
                        [128, 512], F32, tag="acc",
                        name=f"da_{rep}_{cc}_{h}_{ry}_{rx}")
                    cnt[(ry, rx)] = 0
            for qc in range(8):
                for ry in range(2):
                    us = [u for u in range(4) if (u + 1) % 2 == ry]
                    for rx in range(2):
                        vs = [v for v in range(4) if (v + 1) % 2 == rx]
                        for u in us:
                            for v in vs:
                                sy = (ry + 1 - u) // 2
                                sx = (rx + 1 - v) // 2
                                rhs = A_pad[:, qc,
                                            1 + sy + 16 * h: 1 + sy + 16 * h + 16,
                                            1 + sx: 1 + sx + 32]
                                k = cnt[(ry, rx)]
                                nc.tensor.matmul(accs[(ry, rx)][:],
                                                 RW[cc][:, qc, 4 * u + v, :], rhs,
                                                 start=(k == 0), stop=(k == 31))
                                cnt[(ry, rx)] += 1
            for ry in range(2):
                for rx in range(2):
                    dst = out_sb[:, 32 * h + ry: 32 * (h + 1): 2, rx::2]
                    if rx == 0:
                        nc.vector.tensor_scalar_mul(dst, accs[(ry, rx)][:], 0.25)
                    else:
                        nc.scalar.mul(dst, accs[(ry, rx)][:], 0.25)
            eng = nc.sync if (cc + h) % 2 == 0 else nc.scalar
            eng.dma_start(od[:, 32 * h:32 * h + 32],
                          out_sb[:, 32 * h:32 * h + 32])


_NC_CACHE = {}


def _get_nc(cfg=("f32r", "bf16")):
    if cfg not in _NC_CACHE:
        _NC_CACHE[cfg] = build_nc(*cfg)
    return _NC_CACHE[cfg]


def kernel(feature: np.ndarray, mask: np.ndarray) -> np.ndarray:
    feature = np.ascontiguousarray(np.asarray(feature, dtype=np.float32))
    mask = np.asarray(mask, dtype=np.float32)
    nc = _get_nc()
    m0 = np.ascontiguousarray(mask[0, 0])
    in_maps = [{"feature": np.ascontiguousarray(feature[i]), "mask0": m0}
               for i in range(N_CORES)]
    res = run_bass_kernel_spmd(nc, in_maps, list(range(N_CORES)))
    return np.stack([np.asarray(res.results[i]["out"], dtype=np.float32)
                     for i in range(N_CORES)])


# revision 33
# speedup vs baseline: 1.0031x; 1.0031x over previous
"""Contextual-attention kernel for Trainium2, batch-parallel over 8 NeuronCores.

Per core (one image, feature [256,64,64], shared mask [128,128]):
  1. fd = nearest-downsampled feature, zero-padded       [256, 34, 34]
  2. RW deconv patch banks prebuilt early (f-dependent only): PE transposes
     of contiguous-staged (u,v) grids of f_pad2 -> RW[cc][q, c]; PSUM->SBUF
     copies batched 4 transposes at a time, split across DVE/Act.
  3. Gram scores S[q,p] = sum over 9 patch-shift outer products (PE matmuls;
     lhsT from contiguous q-strip staging), scaled by
     rnorm[q] = 1/max(||patch_q||, eps)
  4. fuse conv 1 (diag +-1, row-major) and fuse conv 2 (diag +-1, col-major
     incl. wrap slivers): implemented as PE identity-shift matmuls
     accumulating in PSUM; chunk-boundary single rows via gpsimd
     accumulate-DMAs.  Mask (mm_q) folded into the fuse2 PSUM->SBUF copies.
  5. per-p max via PE transposes (f32r) + free-dim reduce; max row built by
     PE transposes (not DMA); broadcast via 1xK ones matmul; subtract,
     exp(10*x) on ScalarE -> bf16
  6. denominators via ones matmul over q, reciprocal; final weights into
     zero-padded A_pad [q, 34, 34]
  7. deconv: 512 accumulating matmuls vs prebuilt RW -> out[c, parity
     grids]; *0.25; stores split across SP/Act DMA queues.

SBUF: one slot-shared "work" pool (4 x 32KB slots, tag "wk") serves all
large buffers with disjoint lifetimes.
"""
import sys

sys.path.insert(0, "/opt/trn_rl_repo")

import numpy as np

import concourse.bass as bass
import concourse.bacc as bacc_mod
import concourse.mybir as mybir
import concourse.tile as tile
from concourse.masks import make_identity
from concourse.bass_utils import run_bass_kernel_spmd

F32 = mybir.dt.float32
F32R = mybir.dt.float32r
BF16 = mybir.dt.bfloat16
AX = mybir.AxisListType
OP = mybir.AluOpType
ACT = mybir.ActivationFunctionType

N_CORES = 8
C, H, W = 256, 64, 64
SCALE = 10.0
EPS = 1e-4


def build_nc(gram_dt="f32r", dec_dt="bf16", reps=1):
    nc = bacc_mod.Bacc("TRN2", target_bir_lowering=False, debug=False)
    feat = nc.dram_tensor("feature", [C, H, W], F32, kind="ExternalInput")
    mask0 = nc.dram_tensor("mask0", [128, 128], F32, kind="ExternalInput")
    out_d = nc.dram_tensor("out", [C, H, W], F32, kind="ExternalOutput")

    assert dec_dt in ("bf16", "f32")
    ddt = BF16 if dec_dt == "bf16" else F32
    gdt = F32R if gram_dt == "f32r" else F32

    with tile.TileContext(nc) as tc:
        with (
            tc.tile_pool(name="fpool", bufs=1) as fpl,
            tc.tile_pool(name="work", bufs=4) as wk,
            tc.tile_pool(name="wr", bufs=4) as wr,
            tc.tile_pool(name="qbp", bufs=2) as qbp,
            tc.tile_pool(name="sml", bufs=1) as sml,
            tc.tile_pool(name="acc", bufs=4, space="PSUM") as pacc,
            tc.tile_pool(name="ptp", bufs=3, space="PSUM") as ptp,
        ):
            for rep in range(reps):
                _body(nc, tc, fpl, wk, wr, qbp, sml, pacc, ptp,
                      feat, mask0, out_d, gdt, ddt, rep)
    nc.finalize()
    return nc


def _body(nc, tc, fpl, wk, wr, qbp, sml, pacc, ptp, feat, mask0, out_d, gdt, ddt, rep):
    # ---------------- constants ----------------
    ident = sml.tile([128, 128], F32, tag="ident")
    make_identity(nc, ident)
    identr_t = sml.tile([128, 128], F32R, tag="identr")
    nc.vector.tensor_copy(identr_t[:], ident[:])
    identr = identr_t[:]
    ones128 = sml.tile([128, 128], F32, tag="ones128")
    nc.any.memset(ones128[:], 1.0)
    ones_bf = sml.tile([128, 128], BF16, tag="ones_bf")
    nc.any.memset(ones_bf[:], 1.0)
    ident_bq = sml.tile([128, 128], BF16, tag="ident_bf")
    nc.vector.tensor_copy(ident_bq[:], ident[:])
    ident_d = ident_bq if ddt == BF16 else ident
    ones1 = sml.tile([1, 128], F32, tag="ones1")
    nc.any.memset(ones1[:], 1.0)

    # shift matrices for the fuse stages: SH(s)[k, m] = 1 iff k == m + s,
    # so matmul(out, SH(s), rhs) gives out[m] = rhs[m+s] (zero off-range).
    shtmp = sml.tile([128, 128], F32, tag="shtmp")

    def make_shift(name, s, zero_cols=()):
        nc.gpsimd.memset(shtmp[:], 0.0)
        nc.gpsimd.affine_select(
            out=shtmp[:], in_=shtmp[:], compare_op=OP.not_equal, fill=1.0,
            base=-s, pattern=[[-1, 128]], channel_multiplier=1)
        for c in zero_cols:
            nc.gpsimd.memset(shtmp[:, c:c + 1], 0.0)
        t = sml.tile([128, 128], F32R, tag=name)
        nc.vector.tensor_copy(t[:], shtmp[:])
        return t[:]

    sh1p = make_shift("sh1p", 1)
    sh1m = make_shift("sh1m", -1)
    sh32p = make_shift("sh32p", 32)
    sh32m = make_shift("sh32m", -32)
    sh96m = make_shift("sh96m", -96)
    sh96p = make_shift("sh96p", 96)
    # chunk-7 qy=31 wrap: out[m] = rhs[m-95] only for m in 96..126
    w7 = make_shift("w7", -95, zero_cols=(95, 127))
    # chunk-0 qy=0 wrap: out[m] = rhs[m+95] only for m in 1..31
    w0 = make_shift("w0", 95, zero_cols=(0, 32))
    # fuse1 cross-chunk stitch rows: single-element selectors
    st_p = make_shift("st_p", -127)   # out[127] = rhs[0]
    st_m = make_shift("st_m", 127)    # out[0] = rhs[127]

    # ---------------- stage 0: loads & padded layouts ----------------
    # contiguous feature loads on parallel DMA queues (SP cc=0, Act cc=1)
    fraw, fdp = [], []
    for cc in range(2):
        t = wk.tile([128, 64, 64], F32, tag="wk", name=f"fraw_{rep}_{cc}")
        eng = nc.sync if cc == 0 else nc.scalar
        eng.dma_start(t[:], feat[cc * 128:(cc + 1) * 128])
        fraw.append(t)
    zbf = qbp.tile([128, 1156], F32, tag="qb", name=f"zbf_{rep}")
    nc.gpsimd.memset(zbf[:], 0.0)
    for cc in range(2):
        t = fpl.tile([128, 34, 34], gdt, tag=f"fdp_{cc}")
        nc.vector.tensor_copy(t[:].rearrange("p a b -> p (a b)"), zbf[:])
        nc.vector.tensor_copy(t[:, 1:33, 1:33], fraw[cc][:, 0:64:2, 0:64:2])
        fdp.append(t)

    # ---------------- stage 0b: deconv RW banks (built in pieces) --------
    # RW[cc][q-part, qc, u*4+v, c] = f_pad2[c, 2qy+u, 2qx+v] transposed.
    # (u,v) grouped in 4s so one PSUM->SBUF copy covers 4 transposes.
    # Groups are emitted interleaved with the gram/fuse phases to fill
    # PE dependency stalls and spread the DVE/Act copy load.
    RW = [wk.tile([128, 8, 16, 128], ddt, tag="wk", name=f"rw_{rep}_{cc}")
          for cc in range(2)]

    def rw_group(cc, grp):
        rw = RW[cc]
        gbs = []
        for k in range(4):
            uv = grp * 4 + k
            u, v = uv // 4, uv % 4
            gb = wr.tile([128, 1024], ddt, tag="w", name=f"gb_{rep}_{cc}_{uv}")
            gv = gb[:].rearrange("p (a b) -> p a b", a=32)
            # grid rows r(qy) = 2qy+u-1; u=0 -> qy=0 OOB, u=3 -> qy=31 OOB
            y0, y1 = (1, 32) if u == 0 else ((0, 31) if u == 3 else (0, 32))
            x0, x1 = (1, 32) if v == 0 else ((0, 31) if v == 3 else (0, 32))
            if u == 0:
                nc.gpsimd.memset(gv[:, 0, :], 0.0)
            elif u == 3:
                nc.gpsimd.memset(gv[:, 31, :], 0.0)
            if v == 0:
                nc.gpsimd.memset(gv[:, y0:y1, 0], 0.0)
            elif v == 3:
                nc.gpsimd.memset(gv[:, y0:y1, 31], 0.0)
            r0, c0 = 2 * y0 + u - 1, 2 * x0 + v - 1
            r1 = min(r0 + 2 * (y1 - y0), 64)
            c1 = min(c0 + 2 * (x1 - x0), 64)
            nc.scalar.copy(gv[:, y0:y1, x0:x1], fraw[cc][:, r0:r1:2, c0:c1:2])
            gbs.append(gb)
        for qc in range(8):
            ps = ptp.tile([128, 512], ddt, tag="tp")
            for k in range(4):
                nc.tensor.transpose(ps[:, 128 * k:128 * (k + 1)],
                                    gbs[k][:, 128 * qc: 128 * (qc + 1)],
                                    ident_d[:])
            dst = rw[:, qc, 4 * grp: 4 * (grp + 1), :].rearrange(
                "p a b -> p (a b)")
            if qc % 2 == 0:
                nc.vector.tensor_copy(dst, ps[:])
            else:
                nc.scalar.copy(dst, ps[:])

    rw_group(0, 0)
    rw_group(0, 1)

    # ---------------- stage 1: mask -> mm_q [128, 8] ----------------
    msc = sml.tile([1, 3204], F32, tag="msc")
    for k, (dy, dx) in enumerate(((0, 0), (0, 1), (1, 0), (1, 1))):
        off = 0 if k == 0 else 1024
        dst = msc[:, off:off + 1024].rearrange("o (a b) -> o a b", a=32)
        nc.sync.dma_start(dst, mask0[dy::4, dx::4][None])
        if k > 0:
            nc.gpsimd.tensor_add(msc[:, 0:1024], msc[:, 0:1024],
                                 msc[:, 1024:2048])
    msum = msc[:, 0:1024].rearrange("o (a b) -> o a b", a=32)
    mdp = msc[:, 2048:3204].rearrange("o (a b) -> o a b", a=34)
    mbx = msc[:, 1024:2112].rearrange("o (a b) -> o a b", a=34)
    nc.gpsimd.memset(mdp[:], 0.0)
    nc.gpsimd.tensor_scalar(mdp[:, 1:33, 1:33], msum[:], 2.5, None, OP.is_ge)
    nc.gpsimd.tensor_add(mbx[:], mdp[:, :, 0:32], mdp[:, :, 1:33])
    nc.gpsimd.tensor_add(mbx[:], mbx[:], mdp[:, :, 2:34])
    mbox = msc[:, 0:1024].rearrange("o (a b) -> o a b", a=32)
    nc.gpsimd.tensor_add(mbox[:], mbx[:, 0:32, :], mbx[:, 1:33, :])
    nc.gpsimd.tensor_add(mbox[:], mbox[:], mbx[:, 2:34, :])
    mmrow = msc[:, 2112:3136]
    nc.gpsimd.tensor_scalar(mmrow[:].rearrange("o (a b) -> o a b", a=32),
                            mbox[:], 0.0, None, OP.is_equal)
    mm_q = sml.tile([128, 8], F32, tag="mm_q")

    # ---------------- stage 1b: rnorm_q [128, 8] ----------------
    nsc = sml.tile([128, 2244], F32, tag="nsc")
    ssq = nsc[:, 0:1156].rearrange("p (a b) -> p a b", a=34)
    nbx = nsc[:, 1156:2244].rearrange("p (a b) -> p a b", a=34)
    sq = []
    for cc in range(2):
        t = qbp.tile([128, 1156], F32, tag="qb", name=f"sq_{rep}_{cc}")
        nc.scalar.square(t[:], fdp[cc][:].rearrange("p a b -> p (a b)"))
        sq.append(t)
    for (o, n) in ((0, 512), (512, 512), (1024, 132)):
        ps = pacc.tile([128, 512], F32, tag="acc")
        for cc in range(2):
            nc.tensor.matmul(ps[:, :n], ones128[:], sq[cc][:, o:o + n],
                             start=(cc == 0), stop=(cc == 1))
        nc.vector.tensor_copy(nsc[:, o:o + n], ps[:, :n])
    nc.vector.tensor_add(nbx[:], ssq[:, :, 0:32], ssq[:, :, 1:33])
    nc.vector.tensor_add(nbx[:], nbx[:], ssq[:, :, 2:34])
    n2 = nsc[:, 0:1024].rearrange("p (a b) -> p a b", a=32)
    nc.vector.tensor_add(n2[:], nbx[:, 0:32, :], nbx[:, 1:33, :])
    nc.vector.tensor_add(n2[:], n2[:], nbx[:, 2:34, :])
    nrm = nsc[:, 1156:2180]
    rnm = nsc[:, 0:1024]
    # sqrt via exp(0.5*ln(x)): keeps every Act func in one table
    # (natural_log_exp_and_others), avoiding mid-kernel table reloads
    nc.scalar.activation(nrm[:], nsc[:, 0:1024], ACT.Ln)
    nc.scalar.activation(nrm[:], nrm[:], ACT.Exp, bias=0.0, scale=0.5)
    nc.vector.tensor_scalar_max(nrm[:], nrm[:], EPS)
    nc.vector.reciprocal(rnm[:], nrm[:])
    rnorm_q = sml.tile([128, 8], F32, tag="rnorm_q")
    nrm_rep = sml.tile([128, 1024], F32, tag="nrm_rep")

    # ---------------- stage 2: Gram -> M0[q, p] (symmetric) --------------
    # G is symmetric before the rnorm scaling: compute only 256-col blocks
    # (t, g) with g >= t//2; mirror the lower blocks via PE transposes,
    # re-scaled by rnorm[p-part] * ||q||-row (scalar_tensor_tensor).
    # M0/M1 carry one zero pad column on each side of every 1024-wide
    # chunk so every fuse matmul writes a full, aligned 512-wide PSUM slab.
    M0 = wk.tile([128, 8, 1026], F32, tag="wk", name=f"m0_{rep}")
    nc.vector.memset(M0[:, :, 0:1026:1025], 0.0)
    shifts = [(i, j) for i in range(3) for j in range(3)]
    psq = ptp.tile([128, 16], F32, tag="tp", name=f"psq_{rep}")

    def gram_matmuls(t, qb):
        pss = []
        for g in range(t // 2, 4):
            ps = pacc.tile([128, 256], F32, tag="acc")
            k = 0
            for cc in range(2):
                for s, (i, j) in enumerate(shifts):
                    lhsT = qb[:, cc, s, :]
                    rhs = fdp[cc][:, i + 8 * g: i + 8 * g + 8, j:j + 32]
                    nc.tensor.matmul(ps[:], lhsT, rhs,
                                     start=(k == 0), stop=(k == 17))
                    k += 1
            pss.append((g, ps))
        return pss

    def gram_stage(t):
        qb = qbp.tile([128, 2, 9, 128], gdt, tag="qb", name=f"qb_{rep}_{t}")
        for cc in range(2):
            for s, (i, j) in enumerate(shifts):
                nc.vector.tensor_copy(
                    qb[:, cc, s, :].rearrange("p (a b) -> p a b", a=4),
                    fdp[cc][:, i + 4 * t: i + 4 * t + 4, j:j + 32])
        return gram_matmuls(t, qb)

    def gram_finish(t, pss):
        for g, ps in pss:
            nc.vector.tensor_scalar_mul(
                M0[:, t, 1 + 256 * g: 1 + 256 * (g + 1)].bitcast(F32R),
                ps[:], rnorm_q[:, t:t + 1])
        # mirror lower blocks of row t: (t, g) for g < t//2 from rows 2g/2g+1
        for g in range(t // 2):
            psT = ptp.tile([128, 256], F32, tag="tp", name=f"mir_{rep}_{t}_{g}")
            for a in range(2):
                nc.tensor.transpose(
                    psT[:, 128 * a:128 * (a + 1)].bitcast(F32R),
                    M0[:, 2 * g + a, 1 + 128 * t: 1 + 128 * t + 128]
                    .bitcast(F32R),
                    identr)
            nc.vector.scalar_tensor_tensor(
                out=M0[:, t, 1 + 256 * g: 1 + 256 * (g + 1)].bitcast(F32R),
                in0=psT[:], scalar=rnorm_q[:, t:t + 1],
                in1=nrm_rep[:, 256 * g: 256 * (g + 1)],
                op0=OP.mult, op1=OP.mult)

    # t=0 matmuls run on PE while the DVE norm chain drains; the rnorm
    # transposes come after them so the in-order PE queue is never blocked
    pss0 = gram_stage(0)
    for c8 in range(8):
        nc.tensor.transpose(psq[:, 8 + c8:9 + c8],
                            rnm[0:1, 128 * c8:128 * (c8 + 1)], ident[0:1, 0:1])
    nc.vector.tensor_copy(rnorm_q[:], psq[:, 8:16])
    gram_finish(0, pss0)
    for h in range(2):
        psn = pacc.tile([128, 512], F32, tag="acc", name=f"nr_{rep}_{h}")
        nc.tensor.matmul(psn[:], ones1[:], nrm[0:1, 512 * h:512 * (h + 1)],
                         start=True, stop=True)
        nc.vector.tensor_copy(nrm_rep[:, 512 * h:512 * (h + 1)], psn[:])
    for t in range(1, 8):
        pss = gram_stage(t)
        gram_finish(t, pss)
        if t == 1:
            rw_group(0, 2)
        elif t == 3:
            rw_group(0, 3)
        elif t == 2:
            # mask column transport (mask chain surely drained by now)
            for c8 in range(8):
                nc.tensor.transpose(psq[:, c8:c8 + 1],
                                    mmrow[0:1, 128 * c8:128 * (c8 + 1)],
                                    ident[0:1, 0:1])
            nc.vector.tensor_copy(mm_q[:], psq[:, 0:8])

    for grp in range(4):
        rw_group(1, grp)

    # ---------------- stage 3: fuse1 (diag +-1, row-major) on PE ----------
    # M1[q, j] = M0[q, j] + M0[q+1, j+1] + M0[q-1, j-1] (integer q/p index,
    # zero at bounds).  Partition shifts by identity-slice matmuls; the
    # cross-chunk single rows via gpsimd accumulate-DMAs afterwards.
    M1 = wk.tile([128, 8, 1026], F32, tag="wk", name=f"m1_{rep}")
    nc.vector.memset(M1[:, :, 0:1026:1025], 0.0)
    for ch in range(8):
        for h in range(2):
            lo = 512 * h
            ps = pacc.tile([128, 512], F32, tag="acc")
            # center (padded data col = p + 1)
            nc.tensor.matmul(ps[:], identr,
                             M0[:, ch, 1 + lo:1 + lo + 512].bitcast(F32R),
                             start=True, stop=False)
            # +1 term: out[q, p] += M0[q+1, p+1] (p=1023 hits the zero pad)
            nc.tensor.matmul(ps[:], sh1p,
                             M0[:, ch, 2 + lo:2 + lo + 512].bitcast(F32R),
                             start=False, stop=False)
            # -1 term: out[q, p] += M0[q-1, p-1] (p=0 hits the zero pad)
            nc.tensor.matmul(ps[:], sh1m,
                             M0[:, ch, lo:lo + 512].bitcast(F32R),
                             start=False, stop=False)
            # cross-chunk stitch rows via single-element selector matmuls
            if ch < 7:
                nc.tensor.matmul(ps[:], st_p,
                                 M0[:, ch + 1, 2 + lo:2 + lo + 512]
                                 .bitcast(F32R),
                                 start=False, stop=(ch == 0))
            if ch > 0:
                nc.tensor.matmul(ps[:], st_m,
                                 M0[:, ch - 1, lo:lo + 512].bitcast(F32R),
                                 start=False, stop=True)
            if h == 0:
                nc.vector.tensor_copy(
                    M1[:, ch, 1 + lo:1 + lo + 512].bitcast(F32R), ps[:])
            else:
                nc.scalar.copy(M1[:, ch, 1 + lo:1 + lo + 512].bitcast(F32R),
                               ps[:])

    # ---------------- stage 4: fuse2 (diag +-1, col-major) on PE ----------
    # col-major +1 on integer index i (grid (a, b), i = b*32 + a):
    #   q side: q+32 for qy<=30; (qy=31, qx) -> qx+1 (chunk-7 wrap)
    #   p side: p+32 for py<=30; (py=31, px) -> px+1 (free-dim sliver)
    # mask mm_q folded into the PSUM->SBUF copies.
    M2 = wk.tile([128, 8, 1024], F32, tag="wk", name=f"m2_{rep}")

    def m1p(ch_, c0, c1):
        # padded column indexing: data col p lives at padded col p + 1
        return M1[:, ch_, c0:c1].bitcast(F32R)

    for ch in range(8):
        # q-side lhsT for the +1 / -1 col-major terms:
        #   main piece within chunk, cross piece from the adjacent chunk
        qp = [(sh32p, ch), (sh96m, ch + 1) if ch < 7 else (w7, 0)]
        qm = [(sh32m, ch), (sh96p, ch - 1) if ch > 0 else (w0, 7)]
        for h in range(2):
            lo = 512 * h
            ps = pacc.tile([128, 512], F32, tag="acc")
            mm = [(ps[:], identr, m1p(ch, 1 + lo, 1 + lo + 512))]
            sl = []   # sliver matmuls -> aligned scratch psum
            if h == 0:
                # +1: out cols 0..511 (py 0..15) <- src data 32..543
                for l, c in qp:
                    mm.append((ps[:], l, m1p(c, 33, 545)))
                # -1: out cols 32..511 (py 1..15) <- src data 0..479
                for l, c in qm:
                    mm.append((ps[:, 32:512], l, m1p(c, 1, 481)))
                # -1 sliver: out p 1..31 (py=0, px 1..31) <- data 991+p
                for l, c in qm:
                    sl.append((l, m1p(c, 992, 1024)))
                sadd = (1, 32, 1, 32)   # pssl[1:32] -> ps[1:32]
            else:
                # +1 main: out cols 0..479 (py 16..30) <- src data 544..1023
                for l, c in qp:
                    mm.append((ps[:, 0:480], l, m1p(c, 545, 1025)))
                # +1 sliver: out p 992..1022 (py=31, px 0..30) <- data px+1
                for l, c in qp:
                    sl.append((l, m1p(c, 2, 34)))
                sadd = (0, 31, 480, 511)  # pssl[0:31] -> ps[480:511]
                # -1: out cols 512..1023 (py 16..31) <- src data 480..991
                for l, c in qm:
                    mm.append((ps[:], l, m1p(c, 481, 993)))
            for k, (o, l, r) in enumerate(mm):
                nc.tensor.matmul(o, l, r, start=(k == 0),
                                 stop=(k == len(mm) - 1))
            pssl = ptp.tile([128, 32], F32, tag="tp", name=f"sl_{rep}_{ch}_{h}")
            for k, (l, r) in enumerate(sl):
                nc.tensor.matmul(pssl[:], l, r, start=(k == 0),
                                 stop=(k == len(sl) - 1))
            s0, s1, d0, d1 = sadd
            slv = wr.tile([128, 32], F32, tag="w", name=f"slv_{rep}_{ch}_{h}")
            nc.vector.tensor_copy(slv[:, s0:s1], pssl[:, s0:s1])
            nc.vector.tensor_add(ps[:, d0:d1], ps[:, d0:d1], slv[:, s0:s1])
            # masked copy out (fold mm_q)
            if h == 0:
                nc.vector.tensor_scalar_mul(
                    M2[:, ch, lo:lo + 512].bitcast(F32R), ps[:],
                    mm_q[:, ch:ch + 1])
            else:
                nc.scalar.mul(M2[:, ch, lo:lo + 512].bitcast(F32R), ps[:],
                              mm_q[:, ch:ch + 1])

    # ---------------- stage 5: max, subtract, exp ----------------
    mx8 = sml.tile([128, 8, 2], F32, tag="mx8")
    for pt in range(8):
        for g in range(2):
            ps = ptp.tile([128, 512], F32, tag="tp", name=f"tpb_{rep}_{pt}_{g}")
            for t4 in range(4):
                t = 4 * g + t4
                nc.tensor.transpose(
                    ps[:, 128 * t4:128 * (t4 + 1)].bitcast(F32R),
                    M2[:, t, 128 * pt:128 * (pt + 1)].bitcast(F32R),
                    identr)
            nc.vector.reduce_max(mx8[:, pt, g:g + 1], ps[:], axis=AX.X)
    mx_all = sml.tile([128, 8], F32, tag="mx_all")
    for pt in range(8):
        nc.vector.reduce_max(mx_all[:, pt:pt + 1], mx8[:, pt, :], axis=AX.X)
    # max col [128, 8] -> row [1, 1024] via PE transposes
    mxrow = sml.tile([1, 1024], F32, tag="mxrow")
    for g in range(2):
        psr = ptp.tile([1, 512], F32, tag="tp", name=f"mxr_{rep}_{g}")
        for c4 in range(4):
            pt = 4 * g + c4
            nc.tensor.transpose(psr[0:1, 128 * c4:128 * (c4 + 1)],
                                mx_all[:, pt:pt + 1], ident[:])
        nc.vector.tensor_copy(mxrow[:, 512 * g:512 * (g + 1)], psr[:])
    E = wk.tile([128, 8, 1024], BF16, tag="wk", name=f"e_{rep}")
    psrs = []
    bcs = qbp.tile([128, 1024], F32, tag="qb", name=f"bcs_{rep}")
    for h in range(2):
        psr = pacc.tile([128, 512], F32, tag="acc", name=f"bc_{rep}_{h}")
        nc.tensor.matmul(psr[:], ones1[:], mxrow[:, 512 * h:512 * (h + 1)],
                         start=True, stop=True)
        nc.scalar.copy(bcs[:, 512 * h:512 * (h + 1)], psr[:])
        psrs.append(psr)
    # subtract/exp/denominator pipelined at (t, h) granularity; both
    # denominator PSUM groups stay open while exps stream in
    rcp = sml.tile([128, 1024], F32, tag="rcp")
    pssd = [pacc.tile([128, 512], F32, tag="acc", name=f"dn_{rep}_{h}")
            for h in range(2)]
    for t in range(8):
        for h in range(2):
            if h == 0:
                nc.vector.tensor_tensor(
                    M1[:, t, 1 + 512 * h:1 + 512 * (h + 1)].bitcast(F32R),
                    M2[:, t, 512 * h:512 * (h + 1)], psrs[h][:], OP.subtract)
            else:
                nc.gpsimd.tensor_tensor(
                    M1[:, t, 1 + 512 * h:1 + 512 * (h + 1)].bitcast(F32R),
                    M2[:, t, 512 * h:512 * (h + 1)],
                    bcs[:, 512 * h:512 * (h + 1)], OP.subtract)
            nc.scalar.activation(E[:, t, 512 * h:512 * (h + 1)],
                                 M1[:, t, 1 + 512 * h:1 + 512 * (h + 1)],
                                 ACT.Exp, bias=0.0, scale=SCALE)
            nc.tensor.matmul(pssd[h][:], ones_bf[:],
                             E[:, t, 512 * h:512 * (h + 1)],
                             start=(t == 0), stop=(t == 7))
    for h in range(2):
        nc.vector.reciprocal(rcp[:, 512 * h:512 * (h + 1)], pssd[h][:])

    # ---------------- stage 5c: final weights -> A_pad ----------------
    A_pad = wk.tile([128, 8, 34, 34], ddt, tag="wk", name=f"ap_{rep}")
    nc.gpsimd.memset(A_pad[:, :, 0:34:33, :], 0.0)
    nc.gpsimd.memset(A_pad[:, :, 1:33, 0:34:33], 0.0)
    # weight build split into row halves so the top-half deconv can start
    # while bottom-half weights are still being produced
    for (r0, r1) in ((1, 19), (19, 33)):
        for t in range(8):
            nc.vector.scalar_tensor_tensor(
                out=A_pad[:, t, r0:r1, 1:33],
                in0=E[:, t, :].rearrange("p (a b) -> p a b", a=32)
                [:, r0 - 1:r1 - 1, :],
                scalar=mm_q[:, t:t + 1],
                in1=rcp[:].rearrange("p (a b) -> p a b", a=32)
                [:, r0 - 1:r1 - 1, :],
                op0=OP.mult, op1=OP.mult)

    # ---------------- stage 6: deconv ----------------
    for cc in range(2):
        out_sb = wk.tile([128, 64, 64], F32, tag="wk", name=f"os_{rep}_{cc}")
        od = out_d[cc * 128:(cc + 1) * 128]
        for h in range(2):
            accs, cnt = {}, {}
            for ry in range(2):
                for rx in range(2):
                    accs[(ry, rx)] = pacc.tile(
                        [128, 512], F32, tag="acc",
                        name=f"da_{rep}_{cc}_{h}_{ry}_{rx}")
                    cnt[(ry, rx)] = 0
            for qc in range(8):
                for ry in range(2):
                    us = [u for u in range(4) if (u + 1) % 2 == ry]
                    for rx in range(2):
                        vs = [v for v in range(4) if (v + 1) % 2 == rx]
                        for u in us:
                            for v in vs:
                                sy = (ry + 1 - u) // 2
                                sx = (rx + 1 - v) // 2
                                rhs = A_pad[:, qc,
                                            1 + sy + 16 * h: 1 + sy + 16 * h + 16,
                                            1 + sx: 1 + sx + 32]
                                k = cnt[(ry, rx)]
                                nc.tensor.matmul(accs[(ry, rx)][:],
                                                 RW[cc][:, qc, 4 * u + v, :], rhs,
                                                 start=(k == 0), stop=(k == 31))
                                cnt[(ry, rx)] += 1
            for ry in range(2):
                for rx in range(2):
                    dst = out_sb[:, 32 * h + ry: 32 * (h + 1): 2, rx::2]
                    if rx == 0:
                        nc.vector.tensor_scalar_mul(dst, accs[(ry, rx)][:], 0.25)
                    else:
                        nc.scalar.mul(dst, accs[(ry, rx)][:], 0.25)
            eng = nc.sync if (cc + h) % 2 == 0 else nc.scalar
            eng.dma_start(od[:, 32 * h:32 * h + 32],
                          out_sb[:, 32 * h:32 * h + 32])


_NC_CACHE = {}


def _get_nc(cfg=("f32r", "bf16")):
    if cfg not in _NC_CACHE:
        _NC_CACHE[cfg] = build_nc(*cfg)
    return _NC_CACHE[cfg]


def kernel(feature: np.ndarray, mask: np.ndarray) -> np.ndarray:
    feature = np.ascontiguousarray(np.asarray(feature, dtype=np.float32))
    mask = np.asarray(mask, dtype=np.float32)
    nc = _get_nc()
    m0 = np.ascontiguousarray(mask[0, 0])
    in_maps = [{"feature": np.ascontiguousarray(feature[i]), "mask0": m0}
               for i in range(N_CORES)]
    res = run_bass_kernel_spmd(nc, in_maps, list(range(N_CORES)))
    return np.stack([np.asarray(res.results[i]["out"], dtype=np.float32)
                     for i in range(N_CORES)])


# revision 35
# speedup vs baseline: 1.0204x; 1.0172x over previous
"""Contextual-attention kernel for Trainium2, batch-parallel over 8 NeuronCores.

Per core (one image, feature [256,64,64], shared mask [128,128]):
  1. fd = nearest-downsampled feature, zero-padded       [256, 34, 34]
  2. RW deconv patch banks prebuilt early (f-dependent only): PE transposes
     of contiguous-staged (u,v) grids of f_pad2 -> RW[cc][q, c]; PSUM->SBUF
     copies batched 4 transposes at a time, split across DVE/Act.
  3. Gram scores S[q,p] = sum over 9 patch-shift outer products (PE matmuls;
     lhsT from contiguous q-strip staging), scaled by
     rnorm[q] = 1/max(||patch_q||, eps)
  4. fuse conv 1 (diag +-1, row-major) and fuse conv 2 (diag +-1, col-major
     incl. wrap slivers): implemented as PE identity-shift matmuls
     accumulating in PSUM; chunk-boundary single rows via gpsimd
     accumulate-DMAs.  Mask (mm_q) folded into the fuse2 PSUM->SBUF copies.
  5. per-p max via PE transposes (f32r) + free-dim reduce; max row built by
     PE transposes (not DMA); broadcast via 1xK ones matmul; subtract,
     exp(10*x) on ScalarE -> bf16
  6. denominators via ones matmul over q, reciprocal; final weights into
     zero-padded A_pad [q, 34, 34]
  7. deconv: 512 accumulating matmuls vs prebuilt RW -> out[c, parity
     grids]; *0.25; stores split across SP/Act DMA queues.

SBUF: one slot-shared "work" pool (4 x 32KB slots, tag "wk") serves all
large buffers with disjoint lifetimes.
"""
import sys

sys.path.insert(0, "/opt/trn_rl_repo")

import numpy as np

import concourse.bass as bass
import concourse.bacc as bacc_mod
import concourse.mybir as mybir
import concourse.tile as tile
from concourse.masks import make_identity
from concourse.bass_utils import run_bass_kernel_spmd

F32 = mybir.dt.float32
F32R = mybir.dt.float32r
BF16 = mybir.dt.bfloat16
AX = mybir.AxisListType
OP = mybir.AluOpType
ACT = mybir.ActivationFunctionType

N_CORES = 8
C, H, W = 256, 64, 64
SCALE = 10.0
EPS = 1e-4


def build_nc(gram_dt="f32r", dec_dt="bf16", reps=1):
    nc = bacc_mod.Bacc("TRN2", target_bir_lowering=False, debug=False)
    feat = nc.dram_tensor("feature", [C, H, W], F32, kind="ExternalInput")
    mask0 = nc.dram_tensor("mask0", [128, 128], F32, kind="ExternalInput")
    out_d = nc.dram_tensor("out", [C, H, W], F32, kind="ExternalOutput")

    assert dec_dt in ("bf16", "f32")
    ddt = BF16 if dec_dt == "bf16" else F32
    gdt = F32R if gram_dt == "f32r" else F32

    with tile.TileContext(nc) as tc:
        with (
            tc.tile_pool(name="fpool", bufs=1) as fpl,
            tc.tile_pool(name="work", bufs=4) as wk,
            tc.tile_pool(name="wr", bufs=4) as wr,
            tc.tile_pool(name="qbp", bufs=2) as qbp,
            tc.tile_pool(name="sml", bufs=1) as sml,
            tc.tile_pool(name="acc", bufs=4, space="PSUM") as pacc,
            tc.tile_pool(name="ptp", bufs=3, space="PSUM") as ptp,
        ):
            for rep in range(reps):
                _body(nc, tc, fpl, wk, wr, qbp, sml, pacc, ptp,
                      feat, mask0, out_d, gdt, ddt, rep)
    nc.finalize()
    return nc


def _body(nc, tc, fpl, wk, wr, qbp, sml, pacc, ptp, feat, mask0, out_d, gdt, ddt, rep):
    # ---------------- constants ----------------
    ident = sml.tile([128, 128], F32, tag="ident")
    make_identity(nc, ident)
    identr_t = sml.tile([128, 128], F32R, tag="identr")
    nc.vector.tensor_copy(identr_t[:], ident[:])
    identr = identr_t[:]
    ones128 = sml.tile([128, 128], F32, tag="ones128")
    nc.any.memset(ones128[:], 1.0)
    ones_bf = sml.tile([128, 128], BF16, tag="ones_bf")
    nc.any.memset(ones_bf[:], 1.0)
    ident_bq = sml.tile([128, 128], BF16, tag="ident_bf")
    nc.vector.tensor_copy(ident_bq[:], ident[:])
    ident_d = ident_bq if ddt == BF16 else ident
    ones1 = sml.tile([1, 128], F32, tag="ones1")
    nc.any.memset(ones1[:], 1.0)

    # shift matrices for the fuse stages: SH(s)[k, m] = 1 iff k == m + s,
    # so matmul(out, SH(s), rhs) gives out[m] = rhs[m+s] (zero off-range).
    shtmp = sml.tile([128, 128], F32, tag="shtmp")

    def make_shift(name, s, zero_cols=()):
        nc.gpsimd.memset(shtmp[:], 0.0)
        nc.gpsimd.affine_select(
            out=shtmp[:], in_=shtmp[:], compare_op=OP.not_equal, fill=1.0,
            base=-s, pattern=[[-1, 128]], channel_multiplier=1)
        for c in zero_cols:
            nc.gpsimd.memset(shtmp[:, c:c + 1], 0.0)
        t = sml.tile([128, 128], F32R, tag=name)
        nc.vector.tensor_copy(t[:], shtmp[:])
        return t[:]

    sh1p = make_shift("sh1p", 1)
    sh1m = make_shift("sh1m", -1)
    sh32p = make_shift("sh32p", 32)
    sh32m = make_shift("sh32m", -32)
    sh96m = make_shift("sh96m", -96)
    sh96p = make_shift("sh96p", 96)
    # chunk-7 qy=31 wrap: out[m] = rhs[m-95] only for m in 96..126
    w7 = make_shift("w7", -95, zero_cols=(95, 127))
    # chunk-0 qy=0 wrap: out[m] = rhs[m+95] only for m in 1..31
    w0 = make_shift("w0", 95, zero_cols=(0, 32))
    # fuse1 cross-chunk stitch rows: single-element selectors
    st_p = make_shift("st_p", -127)   # out[127] = rhs[0]
    st_m = make_shift("st_m", 127)    # out[0] = rhs[127]

    # ---------------- stage 0: loads & padded layouts ----------------
    # contiguous feature loads on parallel DMA queues (SP cc=0, Act cc=1)
    fraw, fdp = [], []
    for cc in range(2):
        t = wk.tile([128, 64, 64], F32, tag="wk", name=f"fraw_{rep}_{cc}")
        eng = nc.sync if cc == 0 else nc.scalar
        eng.dma_start(t[:], feat[cc * 128:(cc + 1) * 128])
        fraw.append(t)
    zbf = qbp.tile([128, 1156], F32, tag="qb", name=f"zbf_{rep}")
    nc.gpsimd.memset(zbf[:], 0.0)
    for cc in range(2):
        t = fpl.tile([128, 34, 34], gdt, tag=f"fdp_{cc}")
        nc.vector.tensor_copy(t[:].rearrange("p a b -> p (a b)"), zbf[:])
        nc.vector.tensor_copy(t[:, 1:33, 1:33], fraw[cc][:, 0:64:2, 0:64:2])
        fdp.append(t)

    # ---------------- stage 0b: deconv RW banks (built in pieces) --------
    # RW[cc][q-part, qc, u*4+v, c] = f_pad2[c, 2qy+u, 2qx+v] transposed.
    # (u,v) grouped in 4s so one PSUM->SBUF copy covers 4 transposes.
    # Groups are emitted interleaved with the gram/fuse phases to fill
    # PE dependency stalls and spread the DVE/Act copy load.
    RW = [wk.tile([128, 8, 16, 128], ddt, tag="wk", name=f"rw_{rep}_{cc}")
          for cc in range(2)]

    def rw_group(cc, grp):
        rw = RW[cc]
        gbs = []
        for k in range(4):
            uv = grp * 4 + k
            u, v = uv // 4, uv % 4
            gb = wr.tile([128, 1024], ddt, tag="w", name=f"gb_{rep}_{cc}_{uv}")
            gv = gb[:].rearrange("p (a b) -> p a b", a=32)
            # grid rows r(qy) = 2qy+u-1; u=0 -> qy=0 OOB, u=3 -> qy=31 OOB
            y0, y1 = (1, 32) if u == 0 else ((0, 31) if u == 3 else (0, 32))
            x0, x1 = (1, 32) if v == 0 else ((0, 31) if v == 3 else (0, 32))
            if u == 0:
                nc.gpsimd.memset(gv[:, 0, :], 0.0)
            elif u == 3:
                nc.gpsimd.memset(gv[:, 31, :], 0.0)
            if v == 0:
                nc.gpsimd.memset(gv[:, y0:y1, 0], 0.0)
            elif v == 3:
                nc.gpsimd.memset(gv[:, y0:y1, 31], 0.0)
            r0, c0 = 2 * y0 + u - 1, 2 * x0 + v - 1
            r1 = min(r0 + 2 * (y1 - y0), 64)
            c1 = min(c0 + 2 * (x1 - x0), 64)
            nc.scalar.copy(gv[:, y0:y1, x0:x1], fraw[cc][:, r0:r1:2, c0:c1:2])
            gbs.append(gb)
        for qc in range(8):
            ps = ptp.tile([128, 512], ddt, tag="tp")
            for k in range(4):
                nc.tensor.transpose(ps[:, 128 * k:128 * (k + 1)],
                                    gbs[k][:, 128 * qc: 128 * (qc + 1)],
                                    ident_d[:])
            dst = rw[:, qc, 4 * grp: 4 * (grp + 1), :].rearrange(
                "p a b -> p (a b)")
            if qc % 2 == 0:
                nc.vector.tensor_copy(dst, ps[:])
            else:
                nc.scalar.copy(dst, ps[:])

    rw_group(0, 0)
    rw_group(0, 1)

    # ---------------- stage 1: mask -> mm_q [128, 8] ----------------
    msc = sml.tile([1, 3204], F32, tag="msc")
    for k, (dy, dx) in enumerate(((0, 0), (0, 1), (1, 0), (1, 1))):
        off = 0 if k == 0 else 1024
        dst = msc[:, off:off + 1024].rearrange("o (a b) -> o a b", a=32)
        nc.sync.dma_start(dst, mask0[dy::4, dx::4][None])
        if k > 0:
            nc.gpsimd.tensor_add(msc[:, 0:1024], msc[:, 0:1024],
                                 msc[:, 1024:2048])
    msum = msc[:, 0:1024].rearrange("o (a b) -> o a b", a=32)
    mdp = msc[:, 2048:3204].rearrange("o (a b) -> o a b", a=34)
    mbx = msc[:, 1024:2112].rearrange("o (a b) -> o a b", a=34)
    nc.gpsimd.memset(mdp[:], 0.0)
    nc.gpsimd.tensor_scalar(mdp[:, 1:33, 1:33], msum[:], 2.5, None, OP.is_ge)
    nc.gpsimd.tensor_add(mbx[:], mdp[:, :, 0:32], mdp[:, :, 1:33])
    nc.gpsimd.tensor_add(mbx[:], mbx[:], mdp[:, :, 2:34])
    mbox = msc[:, 0:1024].rearrange("o (a b) -> o a b", a=32)
    nc.gpsimd.tensor_add(mbox[:], mbx[:, 0:32, :], mbx[:, 1:33, :])
    nc.gpsimd.tensor_add(mbox[:], mbox[:], mbx[:, 2:34, :])
    mmrow = msc[:, 2112:3136]
    nc.gpsimd.tensor_scalar(mmrow[:].rearrange("o (a b) -> o a b", a=32),
                            mbox[:], 0.0, None, OP.is_equal)
    mm_q = sml.tile([128, 8], F32, tag="mm_q")

    # ---------------- stage 1b: rnorm_q [128, 8] ----------------
    nsc = sml.tile([128, 2244], F32, tag="nsc")
    ssq = nsc[:, 0:1156].rearrange("p (a b) -> p a b", a=34)
    nbx = nsc[:, 1156:2244].rearrange("p (a b) -> p a b", a=34)
    sq = []
    for cc in range(2):
        t = qbp.tile([128, 1156], F32, tag="qb", name=f"sq_{rep}_{cc}")
        nc.scalar.square(t[:], fdp[cc][:].rearrange("p a b -> p (a b)"))
        sq.append(t)
    for (o, n) in ((0, 512), (512, 512), (1024, 132)):
        ps = pacc.tile([128, 512], F32, tag="acc")
        for cc in range(2):
            nc.tensor.matmul(ps[:, :n], ones128[:], sq[cc][:, o:o + n],
                             start=(cc == 0), stop=(cc == 1))
        nc.vector.tensor_copy(nsc[:, o:o + n], ps[:, :n])
    nc.vector.tensor_add(nbx[:], ssq[:, :, 0:32], ssq[:, :, 1:33])
    nc.vector.tensor_add(nbx[:], nbx[:], ssq[:, :, 2:34])
    n2 = nsc[:, 0:1024].rearrange("p (a b) -> p a b", a=32)
    nc.vector.tensor_add(n2[:], nbx[:, 0:32, :], nbx[:, 1:33, :])
    nc.vector.tensor_add(n2[:], n2[:], nbx[:, 2:34, :])
    nrm = nsc[:, 1156:2180]
    rnm = nsc[:, 0:1024]
    # sqrt via exp(0.5*ln(x)): keeps every Act func in one table
    # (natural_log_exp_and_others), avoiding mid-kernel table reloads
    nc.scalar.activation(nrm[:], nsc[:, 0:1024], ACT.Ln)
    nc.scalar.activation(nrm[:], nrm[:], ACT.Exp, bias=0.0, scale=0.5)
    nc.vector.tensor_scalar_max(nrm[:], nrm[:], EPS)
    nc.vector.reciprocal(rnm[:], nrm[:])
    rnorm_q = sml.tile([128, 8], F32, tag="rnorm_q")
    nrm_rep = sml.tile([128, 1024], F32, tag="nrm_rep")

    # ---------------- stage 2: Gram -> M0[q, p] (symmetric) --------------
    # G is symmetric before the rnorm scaling: compute only 256-col blocks
    # (t, g) with g >= t//2; mirror the lower blocks via PE transposes,
    # re-scaled by rnorm[p-part] * ||q||-row (scalar_tensor_tensor).
    # M0/M1 carry one zero pad column on each side of every 1024-wide
    # chunk so every fuse matmul writes a full, aligned 512-wide PSUM slab.
    M0 = wk.tile([128, 8, 1026], F32, tag="wk", name=f"m0_{rep}")
    nc.vector.memset(M0[:, :, 0:1026:1025], 0.0)
    shifts = [(i, j) for i in range(3) for j in range(3)]
    psq = ptp.tile([128, 16], F32, tag="tp", name=f"psq_{rep}")

    def gram_matmuls(t, qb):
        pss = []
        for g in range(t // 2, 4):
            ps = pacc.tile([128, 256], F32, tag="acc")
            k = 0
            for cc in range(2):
                for s, (i, j) in enumerate(shifts):
                    lhsT = qb[:, cc, s, :]
                    rhs = fdp[cc][:, i + 8 * g: i + 8 * g + 8, j:j + 32]
                    nc.tensor.matmul(ps[:], lhsT, rhs,
                                     start=(k == 0), stop=(k == 17))
                    k += 1
            pss.append((g, ps))
        return pss

    def gram_stage(t):
        qb = qbp.tile([128, 2, 9, 128], gdt, tag="qb", name=f"qb_{rep}_{t}")
        for cc in range(2):
            for s, (i, j) in enumerate(shifts):
                nc.vector.tensor_copy(
                    qb[:, cc, s, :].rearrange("p (a b) -> p a b", a=4),
                    fdp[cc][:, i + 4 * t: i + 4 * t + 4, j:j + 32])
        return gram_matmuls(t, qb)

    def gram_finish(t, pss):
        for g, ps in pss:
            nc.vector.tensor_scalar_mul(
                M0[:, t, 1 + 256 * g: 1 + 256 * (g + 1)].bitcast(F32R),
                ps[:], rnorm_q[:, t:t + 1])
        # mirror lower blocks of row t: (t, g) for g < t//2 from rows 2g/2g+1
        for g in range(t // 2):
            psT = ptp.tile([128, 256], F32, tag="tp", name=f"mir_{rep}_{t}_{g}")
            for a in range(2):
                nc.tensor.transpose(
                    psT[:, 128 * a:128 * (a + 1)].bitcast(F32R),
                    M0[:, 2 * g + a, 1 + 128 * t: 1 + 128 * t + 128]
                    .bitcast(F32R),
                    identr)
            nc.vector.scalar_tensor_tensor(
                out=M0[:, t, 1 + 256 * g: 1 + 256 * (g + 1)].bitcast(F32R),
                in0=psT[:], scalar=rnorm_q[:, t:t + 1],
                in1=nrm_rep[:, 256 * g: 256 * (g + 1)],
                op0=OP.mult, op1=OP.mult)

    # t=0 matmuls run on PE while the DVE norm chain drains; the rnorm
    # transposes come after them so the in-order PE queue is never blocked
    pss0 = gram_stage(0)
    for c8 in range(8):
        nc.tensor.transpose(psq[:, 8 + c8:9 + c8],
                            rnm[0:1, 128 * c8:128 * (c8 + 1)], ident[0:1, 0:1])
    nc.vector.tensor_copy(rnorm_q[:], psq[:, 8:16])
    gram_finish(0, pss0)
    for h in range(2):
        psn = pacc.tile([128, 512], F32, tag="acc", name=f"nr_{rep}_{h}")
        nc.tensor.matmul(psn[:], ones1[:], nrm[0:1, 512 * h:512 * (h + 1)],
                         start=True, stop=True)
        nc.vector.tensor_copy(nrm_rep[:, 512 * h:512 * (h + 1)], psn[:])
    for t in range(1, 8):
        pss = gram_stage(t)
        gram_finish(t, pss)
        if t == 1:
            rw_group(0, 2)
        elif t == 3:
            rw_group(0, 3)
        elif t == 2:
            # mask column transport (mask chain surely drained by now)
            for c8 in range(8):
                nc.tensor.transpose(psq[:, c8:c8 + 1],
                                    mmrow[0:1, 128 * c8:128 * (c8 + 1)],
                                    ident[0:1, 0:1])
            nc.vector.tensor_copy(mm_q[:], psq[:, 0:8])

    for grp in range(4):
        rw_group(1, grp)

    # ---------------- stage 3: fuse1 (diag +-1, row-major) on PE ----------
    # M1[q, j] = M0[q, j] + M0[q+1, j+1] + M0[q-1, j-1] (integer q/p index,
    # zero at bounds).  Partition shifts by identity-slice matmuls; the
    # cross-chunk single rows via gpsimd accumulate-DMAs afterwards.
    M1 = wk.tile([128, 8, 1026], F32, tag="wk", name=f"m1_{rep}")
    nc.vector.memset(M1[:, :, 0:1026:1025], 0.0)
    for ch in range(8):
        for h in range(2):
            lo = 512 * h
            ps = pacc.tile([128, 512], F32, tag="acc")
            # center (padded data col = p + 1)
            nc.tensor.matmul(ps[:], identr,
                             M0[:, ch, 1 + lo:1 + lo + 512].bitcast(F32R),
                             start=True, stop=False)
            # +1 term: out[q, p] += M0[q+1, p+1] (p=1023 hits the zero pad)
            nc.tensor.matmul(ps[:], sh1p,
                             M0[:, ch, 2 + lo:2 + lo + 512].bitcast(F32R),
                             start=False, stop=False)
            # -1 term: out[q, p] += M0[q-1, p-1] (p=0 hits the zero pad)
            nc.tensor.matmul(ps[:], sh1m,
                             M0[:, ch, lo:lo + 512].bitcast(F32R),
                             start=False, stop=False)
            # cross-chunk stitch rows via single-element selector matmuls
            if ch < 7:
                nc.tensor.matmul(ps[:], st_p,
                                 M0[:, ch + 1, 2 + lo:2 + lo + 512]
                                 .bitcast(F32R),
                                 start=False, stop=(ch == 0))
            if ch > 0:
                nc.tensor.matmul(ps[:], st_m,
                                 M0[:, ch - 1, lo:lo + 512].bitcast(F32R),
                                 start=False, stop=True)
            if h == 0:
                nc.vector.tensor_copy(
                    M1[:, ch, 1 + lo:1 + lo + 512].bitcast(F32R), ps[:])
            else:
                nc.scalar.copy(M1[:, ch, 1 + lo:1 + lo + 512].bitcast(F32R),
                               ps[:])

    # ---------------- stage 4: fuse2 (diag +-1, col-major) on PE ----------
    # col-major +1 on integer index i (grid (a, b), i = b*32 + a):
    #   q side: q+32 for qy<=30; (qy=31, qx) -> qx+1 (chunk-7 wrap)
    #   p side: p+32 for py<=30; (py=31, px) -> px+1 (free-dim sliver)
    # mask mm_q folded into the PSUM->SBUF copies.
    M2 = wk.tile([128, 8, 1024], F32, tag="wk", name=f"m2_{rep}")

    def m1p(ch_, c0, c1):
        # padded column indexing: data col p lives at padded col p + 1
        return M1[:, ch_, c0:c1].bitcast(F32R)

    for ch in range(8):
        # q-side lhsT for the +1 / -1 col-major terms:
        #   main piece within chunk, cross piece from the adjacent chunk
        qp = [(sh32p, ch), (sh96m, ch + 1) if ch < 7 else (w7, 0)]
        qm = [(sh32m, ch), (sh96p, ch - 1) if ch > 0 else (w0, 7)]
        for h in range(2):
            lo = 512 * h
            ps = pacc.tile([128, 512], F32, tag="acc")
            mm = [(ps[:], identr, m1p(ch, 1 + lo, 1 + lo + 512))]
            sl = []   # sliver matmuls -> aligned scratch psum
            if h == 0:
                # +1: out cols 0..511 (py 0..15) <- src data 32..543
                for l, c in qp:
                    mm.append((ps[:], l, m1p(c, 33, 545)))
                # -1: out cols 32..511 (py 1..15) <- src data 0..479
                for l, c in qm:
                    mm.append((ps[:, 32:512], l, m1p(c, 1, 481)))
                # -1 sliver: out p 1..31 (py=0, px 1..31) <- data 991+p
                for l, c in qm:
                    sl.append((l, m1p(c, 992, 1024)))
                sadd = (1, 32, 1, 32)   # pssl[1:32] -> ps[1:32]
            else:
                # +1 main: out cols 0..479 (py 16..30) <- src data 544..1023
                for l, c in qp:
                    mm.append((ps[:, 0:480], l, m1p(c, 545, 1025)))
                # +1 sliver: out p 992..1022 (py=31, px 0..30) <- data px+1
                for l, c in qp:
                    sl.append((l, m1p(c, 2, 34)))
                sadd = (0, 31, 480, 511)  # pssl[0:31] -> ps[480:511]
                # -1: out cols 512..1023 (py 16..31) <- src data 480..991
                for l, c in qm:
                    mm.append((ps[:], l, m1p(c, 481, 993)))
            for k, (o, l, r) in enumerate(mm):
                nc.tensor.matmul(o, l, r, start=(k == 0),
                                 stop=(k == len(mm) - 1))
            pssl = ptp.tile([128, 32], F32, tag="tp", name=f"sl_{rep}_{ch}_{h}")
            for k, (l, r) in enumerate(sl):
                nc.tensor.matmul(pssl[:], l, r, start=(k == 0),
                                 stop=(k == len(sl) - 1))
            s0, s1, d0, d1 = sadd
            slv = wr.tile([128, 32], F32, tag="w", name=f"slv_{rep}_{ch}_{h}")
            nc.vector.tensor_copy(slv[:, s0:s1], pssl[:, s0:s1])
            nc.vector.tensor_add(ps[:, d0:d1], ps[:, d0:d1], slv[:, s0:s1])
            # masked copy out (fold mm_q)
            if h == 0:
                nc.vector.tensor_scalar_mul(
                    M2[:, ch, lo:lo + 512].bitcast(F32R), ps[:],
                    mm_q[:, ch:ch + 1])
            else:
                nc.scalar.mul(M2[:, ch, lo:lo + 512].bitcast(F32R), ps[:],
                              mm_q[:, ch:ch + 1])

    # ---------------- stage 5: max, subtract, exp ----------------
    mx8 = sml.tile([128, 8, 2], F32, tag="mx8")
    for pt in range(8):
        for g in range(2):
            ps = ptp.tile([128, 512], F32, tag="tp", name=f"tpb_{rep}_{pt}_{g}")
            for t4 in range(4):
                t = 4 * g + t4
                nc.tensor.transpose(
                    ps[:, 128 * t4:128 * (t4 + 1)].bitcast(F32R),
                    M2[:, t, 128 * pt:128 * (pt + 1)].bitcast(F32R),
                    identr)
            nc.vector.reduce_max(mx8[:, pt, g:g + 1], ps[:], axis=AX.X)
    mx_all = sml.tile([128, 8], F32, tag="mx_all")
    for pt in range(8):
        nc.vector.reduce_max(mx_all[:, pt:pt + 1], mx8[:, pt, :], axis=AX.X)
    # max col [128, 8] -> row [1, 1024] via PE transposes
    mxrow = sml.tile([1, 1024], F32, tag="mxrow")
    for g in range(2):
        psr = ptp.tile([1, 512], F32, tag="tp", name=f"mxr_{rep}_{g}")
        for c4 in range(4):
            pt = 4 * g + c4
            nc.tensor.transpose(psr[0:1, 128 * c4:128 * (c4 + 1)],
                                mx_all[:, pt:pt + 1], ident[:])
        nc.vector.tensor_copy(mxrow[:, 512 * g:512 * (g + 1)], psr[:])
    E = wk.tile([128, 8, 1024], BF16, tag="wk", name=f"e_{rep}")
    psrs = []
    bcs = qbp.tile([128, 1024], F32, tag="qb", name=f"bcs_{rep}")
    for h in range(2):
        psr = pacc.tile([128, 512], F32, tag="acc", name=f"bc_{rep}_{h}")
        nc.tensor.matmul(psr[:], ones1[:], mxrow[:, 512 * h:512 * (h + 1)],
                         start=True, stop=True)
        nc.scalar.copy(bcs[:, 512 * h:512 * (h + 1)], psr[:])
        psrs.append(psr)
    # subtract/exp/denominator pipelined at (t, h) granularity; both
    # denominator PSUM groups stay open while exps stream in
    rcp = sml.tile([128, 1024], F32, tag="rcp")
    pssd = [pacc.tile([128, 512], F32, tag="acc", name=f"dn_{rep}_{h}")
            for h in range(2)]
    for t in range(8):
        for h in range(2):
            st = wr.tile([128, 512], F32, tag="w", name=f"st_{rep}_{t}_{h}")
            if h == 0:
                nc.vector.tensor_tensor(
                    st[:], M2[:, t, 512 * h:512 * (h + 1)], psrs[h][:],
                    OP.subtract)
            else:
                nc.gpsimd.tensor_tensor(
                    st[:], M2[:, t, 512 * h:512 * (h + 1)],
                    bcs[:, 512 * h:512 * (h + 1)], OP.subtract)
            nc.scalar.activation(E[:, t, 512 * h:512 * (h + 1)], st[:],
                                 ACT.Exp, bias=0.0, scale=SCALE)
            nc.tensor.matmul(pssd[h][:], ones_bf[:],
                             E[:, t, 512 * h:512 * (h + 1)],
                             start=(t == 0), stop=(t == 7))
    for h in range(2):
        nc.vector.reciprocal(rcp[:, 512 * h:512 * (h + 1)], pssd[h][:])

    # ---------------- stage 5c: final weights -> A_pad ----------------
    A_pad = wk.tile([128, 8, 34, 34], ddt, tag="wk", name=f"ap_{rep}")
    nc.gpsimd.memset(A_pad[:, :, 0:34:33, :], 0.0)
    nc.gpsimd.memset(A_pad[:, :, 1:33, 0:34:33], 0.0)
    # weight build split into row halves so the top-half deconv can start
    # while bottom-half weights are still being produced
    for (r0, r1) in ((1, 19), (19, 33)):
        for t in range(8):
            nc.vector.scalar_tensor_tensor(
                out=A_pad[:, t, r0:r1, 1:33],
                in0=E[:, t, :].rearrange("p (a b) -> p a b", a=32)
                [:, r0 - 1:r1 - 1, :],
                scalar=mm_q[:, t:t + 1],
                in1=rcp[:].rearrange("p (a b) -> p a b", a=32)
                [:, r0 - 1:r1 - 1, :],
                op0=OP.mult, op1=OP.mult)

    # ---------------- stage 6: deconv ----------------
    for cc in range(2):
        out_sb = wk.tile([128, 64, 64], F32, tag="wk", name=f"os_{rep}_{cc}")
        od = out_d[cc * 128:(cc + 1) * 128]
        for h in range(2):
            accs, cnt = {}, {}
            for ry in range(2):
                for rx in range(2):
                    accs[(ry, rx)] = pacc.tile(
                        [128, 512], F32, tag="acc",
                        name=f"da_{rep}_{cc}_{h}_{ry}_{rx}")
                    cnt[(ry, rx)] = 0
            for qc in range(8):
                for ry in range(2):
                    us = [u for u in range(4) if (u + 1) % 2 == ry]
                    for rx in range(2):
                        vs = [v for v in range(4) if (v + 1) % 2 == rx]
                        for u in us:
                            for v in vs:
                                sy = (ry + 1 - u) // 2
                                sx = (rx + 1 - v) // 2
                                rhs = A_pad[:, qc,
                                            1 + sy + 16 * h: 1 + sy + 16 * h + 16,
                                            1 + sx: 1 + sx + 32]
                                k = cnt[(ry, rx)]
                                nc.tensor.matmul(accs[(ry, rx)][:],
                                                 RW[cc][:, qc, 4 * u + v, :], rhs,
                                                 start=(k == 0), stop=(k == 31))
                                cnt[(ry, rx)] += 1
            for ry in range(2):
                for rx in range(2):
                    dst = out_sb[:, 32 * h + ry: 32 * (h + 1): 2, rx::2]
                    if rx == 0:
                        nc.vector.tensor_scalar_mul(dst, accs[(ry, rx)][:], 0.25)
                    else:
                        nc.scalar.mul(dst, accs[(ry, rx)][:], 0.25)
            eng = nc.sync if (cc + h) % 2 == 0 else nc.scalar
            eng.dma_start(od[:, 32 * h:32 * h + 32],
                          out_sb[:, 32 * h:32 * h + 32])


_NC_CACHE = {}


def _get_nc(cfg=("f32r", "bf16")):
    if cfg not in _NC_CACHE:
        _NC_CACHE[cfg] = build_nc(*cfg)
    return _NC_CACHE[cfg]


def kernel(feature: np.ndarray, mask: np.ndarray) -> np.ndarray:
    feature = np.ascontiguousarray(np.asarray(feature, dtype=np.float32))
    mask = np.asarray(mask, dtype=np.float32)
    nc = _get_nc()
    m0 = np.ascontiguousarray(mask[0, 0])
    in_maps = [{"feature": np.ascontiguousarray(feature[i]), "mask0": m0}
               for i in range(N_CORES)]
    res = run_bass_kernel_spmd(nc, in_maps, list(range(N_CORES)))
    return np.stack([np.asarray(res.results[i]["out"], dtype=np.float32)
                     for i in range(N_CORES)])


# revision 36
# speedup vs baseline: 1.0269x; 1.0064x over previous
"""Contextual-attention kernel for Trainium2, batch-parallel over 8 NeuronCores.

Per core (one image, feature [256,64,64], shared mask [128,128]):
  1. fd = nearest-downsampled feature, zero-padded       [256, 34, 34]
  2. RW deconv patch banks prebuilt early (f-dependent only): PE transposes
     of contiguous-staged (u,v) grids of f_pad2 -> RW[cc][q, c]; PSUM->SBUF
     copies batched 4 transposes at a time, split across DVE/Act.
  3. Gram scores S[q,p] = sum over 9 patch-shift outer products (PE matmuls;
     lhsT from contiguous q-strip staging), scaled by
     rnorm[q] = 1/max(||patch_q||, eps)
  4. fuse conv 1 (diag +-1, row-major) and fuse conv 2 (diag +-1, col-major
     incl. wrap slivers): implemented as PE identity-shift matmuls
     accumulating in PSUM; chunk-boundary single rows via gpsimd
     accumulate-DMAs.  Mask (mm_q) folded into the fuse2 PSUM->SBUF copies.
  5. per-p max via PE transposes (f32r) + free-dim reduce; max row built by
     PE transposes (not DMA); broadcast via 1xK ones matmul; subtract,
     exp(10*x) on ScalarE -> bf16
  6. denominators via ones matmul over q, reciprocal; final weights into
     zero-padded A_pad [q, 34, 34]
  7. deconv: 512 accumulating matmuls vs prebuilt RW -> out[c, parity
     grids]; *0.25; stores split across SP/Act DMA queues.

SBUF: one slot-shared "work" pool (4 x 32KB slots, tag "wk") serves all
large buffers with disjoint lifetimes.
"""
import sys

sys.path.insert(0, "/opt/trn_rl_repo")

import numpy as np

import concourse.bass as bass
import concourse.bacc as bacc_mod
import concourse.mybir as mybir
import concourse.tile as tile
from concourse.masks import make_identity
from concourse.bass_utils import run_bass_kernel_spmd

F32 = mybir.dt.float32
F32R = mybir.dt.float32r
BF16 = mybir.dt.bfloat16
AX = mybir.AxisListType
OP = mybir.AluOpType
ACT = mybir.ActivationFunctionType

N_CORES = 8
C, H, W = 256, 64, 64
SCALE = 10.0
EPS = 1e-4


def build_nc(gram_dt="f32r", dec_dt="bf16", reps=1):
    nc = bacc_mod.Bacc("TRN2", target_bir_lowering=False, debug=False)
    feat = nc.dram_tensor("feature", [C, H, W], F32, kind="ExternalInput")
    mask0 = nc.dram_tensor("mask0", [128, 128], F32, kind="ExternalInput")
    out_d = nc.dram_tensor("out", [C, H, W], F32, kind="ExternalOutput")

    assert dec_dt in ("bf16", "f32")
    ddt = BF16 if dec_dt == "bf16" else F32
    gdt = F32R if gram_dt == "f32r" else F32

    with tile.TileContext(nc) as tc:
        with (
            tc.tile_pool(name="fpool", bufs=1) as fpl,
            tc.tile_pool(name="work", bufs=4) as wk,
            tc.tile_pool(name="wr", bufs=4) as wr,
            tc.tile_pool(name="qbp", bufs=2) as qbp,
            tc.tile_pool(name="sml", bufs=1) as sml,
            tc.tile_pool(name="acc", bufs=5, space="PSUM") as pacc,
            tc.tile_pool(name="ptp", bufs=3, space="PSUM") as ptp,
        ):
            for rep in range(reps):
                _body(nc, tc, fpl, wk, wr, qbp, sml, pacc, ptp,
                      feat, mask0, out_d, gdt, ddt, rep)
    nc.finalize()
    return nc


def _body(nc, tc, fpl, wk, wr, qbp, sml, pacc, ptp, feat, mask0, out_d, gdt, ddt, rep):
    # ---------------- constants ----------------
    ident = sml.tile([128, 128], F32, tag="ident")
    make_identity(nc, ident)
    identr_t = sml.tile([128, 128], F32R, tag="identr")
    nc.vector.tensor_copy(identr_t[:], ident[:])
    identr = identr_t[:]
    ones128 = sml.tile([128, 128], F32, tag="ones128")
    nc.any.memset(ones128[:], 1.0)
    ones_bf = sml.tile([128, 128], BF16, tag="ones_bf")
    nc.any.memset(ones_bf[:], 1.0)
    ident_bq = sml.tile([128, 128], BF16, tag="ident_bf")
    nc.vector.tensor_copy(ident_bq[:], ident[:])
    ident_d = ident_bq if ddt == BF16 else ident
    ones1 = sml.tile([1, 128], F32, tag="ones1")
    nc.any.memset(ones1[:], 1.0)

    # shift matrices for the fuse stages: SH(s)[k, m] = 1 iff k == m + s,
    # so matmul(out, SH(s), rhs) gives out[m] = rhs[m+s] (zero off-range).
    shtmp = sml.tile([128, 128], F32, tag="shtmp")

    def make_shift(name, s, zero_cols=()):
        nc.gpsimd.memset(shtmp[:], 0.0)
        nc.gpsimd.affine_select(
            out=shtmp[:], in_=shtmp[:], compare_op=OP.not_equal, fill=1.0,
            base=-s, pattern=[[-1, 128]], channel_multiplier=1)
        for c in zero_cols:
            nc.gpsimd.memset(shtmp[:, c:c + 1], 0.0)
        t = sml.tile([128, 128], F32R, tag=name)
        nc.vector.tensor_copy(t[:], shtmp[:])
        return t[:]

    sh1p = make_shift("sh1p", 1)
    sh1m = make_shift("sh1m", -1)
    sh32p = make_shift("sh32p", 32)
    sh32m = make_shift("sh32m", -32)
    sh96m = make_shift("sh96m", -96)
    sh96p = make_shift("sh96p", 96)
    # chunk-7 qy=31 wrap: out[m] = rhs[m-95] only for m in 96..126
    w7 = make_shift("w7", -95, zero_cols=(95, 127))
    # chunk-0 qy=0 wrap: out[m] = rhs[m+95] only for m in 1..31
    w0 = make_shift("w0", 95, zero_cols=(0, 32))
    # fuse1 cross-chunk stitch rows: single-element selectors
    st_p = make_shift("st_p", -127)   # out[127] = rhs[0]
    st_m = make_shift("st_m", 127)    # out[0] = rhs[127]

    # ---------------- stage 0: loads & padded layouts ----------------
    # contiguous feature loads on parallel DMA queues (SP cc=0, Act cc=1)
    fraw, fdp = [], []
    for cc in range(2):
        t = wk.tile([128, 64, 64], F32, tag="wk", name=f"fraw_{rep}_{cc}")
        eng = nc.sync if cc == 0 else nc.scalar
        eng.dma_start(t[:], feat[cc * 128:(cc + 1) * 128])
        fraw.append(t)
    zbf = qbp.tile([128, 1156], F32, tag="qb", name=f"zbf_{rep}")
    nc.gpsimd.memset(zbf[:], 0.0)
    for cc in range(2):
        t = fpl.tile([128, 34, 34], gdt, tag=f"fdp_{cc}")
        nc.vector.tensor_copy(t[:].rearrange("p a b -> p (a b)"), zbf[:])
        nc.vector.tensor_copy(t[:, 1:33, 1:33], fraw[cc][:, 0:64:2, 0:64:2])
        fdp.append(t)

    # ---------------- stage 0b: deconv RW banks (built in pieces) --------
    # RW[cc][q-part, qc, u*4+v, c] = f_pad2[c, 2qy+u, 2qx+v] transposed.
    # (u,v) grouped in 4s so one PSUM->SBUF copy covers 4 transposes.
    # Groups are emitted interleaved with the gram/fuse phases to fill
    # PE dependency stalls and spread the DVE/Act copy load.
    RW = [wk.tile([128, 8, 16, 128], ddt, tag="wk", name=f"rw_{rep}_{cc}")
          for cc in range(2)]

    def rw_group(cc, grp):
        rw = RW[cc]
        gbs = []
        for k in range(4):
            uv = grp * 4 + k
            u, v = uv // 4, uv % 4
            gb = wr.tile([128, 1024], ddt, tag="w", name=f"gb_{rep}_{cc}_{uv}")
            gv = gb[:].rearrange("p (a b) -> p a b", a=32)
            # grid rows r(qy) = 2qy+u-1; u=0 -> qy=0 OOB, u=3 -> qy=31 OOB
            y0, y1 = (1, 32) if u == 0 else ((0, 31) if u == 3 else (0, 32))
            x0, x1 = (1, 32) if v == 0 else ((0, 31) if v == 3 else (0, 32))
            if u == 0:
                nc.gpsimd.memset(gv[:, 0, :], 0.0)
            elif u == 3:
                nc.gpsimd.memset(gv[:, 31, :], 0.0)
            if v == 0:
                nc.gpsimd.memset(gv[:, y0:y1, 0], 0.0)
            elif v == 3:
                nc.gpsimd.memset(gv[:, y0:y1, 31], 0.0)
            r0, c0 = 2 * y0 + u - 1, 2 * x0 + v - 1
            r1 = min(r0 + 2 * (y1 - y0), 64)
            c1 = min(c0 + 2 * (x1 - x0), 64)
            nc.scalar.copy(gv[:, y0:y1, x0:x1], fraw[cc][:, r0:r1:2, c0:c1:2])
            gbs.append(gb)
        for qc in range(8):
            ps = ptp.tile([128, 512], ddt, tag="tp")
            for k in range(4):
                nc.tensor.transpose(ps[:, 128 * k:128 * (k + 1)],
                                    gbs[k][:, 128 * qc: 128 * (qc + 1)],
                                    ident_d[:])
            dst = rw[:, qc, 4 * grp: 4 * (grp + 1), :].rearrange(
                "p a b -> p (a b)")
            if qc % 2 == 0:
                nc.vector.tensor_copy(dst, ps[:])
            else:
                nc.scalar.copy(dst, ps[:])

    rw_group(0, 0)
    rw_group(0, 1)

    # ---------------- stage 1: mask -> mm_q [128, 8] ----------------
    msc = sml.tile([1, 3204], F32, tag="msc")
    for k, (dy, dx) in enumerate(((0, 0), (0, 1), (1, 0), (1, 1))):
        off = 0 if k == 0 else 1024
        dst = msc[:, off:off + 1024].rearrange("o (a b) -> o a b", a=32)
        nc.sync.dma_start(dst, mask0[dy::4, dx::4][None])
        if k > 0:
            nc.gpsimd.tensor_add(msc[:, 0:1024], msc[:, 0:1024],
                                 msc[:, 1024:2048])
    msum = msc[:, 0:1024].rearrange("o (a b) -> o a b", a=32)
    mdp = msc[:, 2048:3204].rearrange("o (a b) -> o a b", a=34)
    mbx = msc[:, 1024:2112].rearrange("o (a b) -> o a b", a=34)
    nc.gpsimd.memset(mdp[:], 0.0)
    nc.gpsimd.tensor_scalar(mdp[:, 1:33, 1:33], msum[:], 2.5, None, OP.is_ge)
    nc.gpsimd.tensor_add(mbx[:], mdp[:, :, 0:32], mdp[:, :, 1:33])
    nc.gpsimd.tensor_add(mbx[:], mbx[:], mdp[:, :, 2:34])
    mbox = msc[:, 0:1024].rearrange("o (a b) -> o a b", a=32)
    nc.gpsimd.tensor_add(mbox[:], mbx[:, 0:32, :], mbx[:, 1:33, :])
    nc.gpsimd.tensor_add(mbox[:], mbox[:], mbx[:, 2:34, :])
    mmrow = msc[:, 2112:3136]
    nc.gpsimd.tensor_scalar(mmrow[:].rearrange("o (a b) -> o a b", a=32),
                            mbox[:], 0.0, None, OP.is_equal)
    mm_q = sml.tile([128, 8], F32, tag="mm_q")

    # ---------------- stage 1b: rnorm_q [128, 8] ----------------
    nsc = sml.tile([128, 2244], F32, tag="nsc")
    ssq = nsc[:, 0:1156].rearrange("p (a b) -> p a b", a=34)
    nbx = nsc[:, 1156:2244].rearrange("p (a b) -> p a b", a=34)
    sq = []
    for cc in range(2):
        t = qbp.tile([128, 1156], F32, tag="qb", name=f"sq_{rep}_{cc}")
        nc.scalar.square(t[:], fdp[cc][:].rearrange("p a b -> p (a b)"))
        sq.append(t)
    for (o, n) in ((0, 512), (512, 512), (1024, 132)):
        ps = pacc.tile([128, 512], F32, tag="acc")
        for cc in range(2):
            nc.tensor.matmul(ps[:, :n], ones128[:], sq[cc][:, o:o + n],
                             start=(cc == 0), stop=(cc == 1))
        nc.vector.tensor_copy(nsc[:, o:o + n], ps[:, :n])
    nc.vector.tensor_add(nbx[:], ssq[:, :, 0:32], ssq[:, :, 1:33])
    nc.vector.tensor_add(nbx[:], nbx[:], ssq[:, :, 2:34])
    n2 = nsc[:, 0:1024].rearrange("p (a b) -> p a b", a=32)
    nc.vector.tensor_add(n2[:], nbx[:, 0:32, :], nbx[:, 1:33, :])
    nc.vector.tensor_add(n2[:], n2[:], nbx[:, 2:34, :])
    nrm = nsc[:, 1156:2180]
    rnm = nsc[:, 0:1024]
    # sqrt via exp(0.5*ln(x)): keeps every Act func in one table
    # (natural_log_exp_and_others), avoiding mid-kernel table reloads
    nc.scalar.activation(nrm[:], nsc[:, 0:1024], ACT.Ln)
    nc.scalar.activation(nrm[:], nrm[:], ACT.Exp, bias=0.0, scale=0.5)
    nc.vector.tensor_scalar_max(nrm[:], nrm[:], EPS)
    nc.vector.reciprocal(rnm[:], nrm[:])
    rnorm_q = sml.tile([128, 8], F32, tag="rnorm_q")
    nrm_rep = sml.tile([128, 1024], F32, tag="nrm_rep")

    # ---------------- stage 2: Gram -> M0[q, p] (symmetric) --------------
    # G is symmetric before the rnorm scaling: compute only 256-col blocks
    # (t, g) with g >= t//2; mirror the lower blocks via PE transposes,
    # re-scaled by rnorm[p-part] * ||q||-row (scalar_tensor_tensor).
    # M0/M1 carry one zero pad column on each side of every 1024-wide
    # chunk so every fuse matmul writes a full, aligned 512-wide PSUM slab.
    M0 = wk.tile([128, 8, 1026], F32, tag="wk", name=f"m0_{rep}")
    nc.vector.memset(M0[:, :, 0:1026:1025], 0.0)
    shifts = [(i, j) for i in range(3) for j in range(3)]
    psq = ptp.tile([128, 16], F32, tag="tp", name=f"psq_{rep}")

    def gram_matmuls(t, qb):
        pss = []
        for g in range(t // 2, 4):
            ps = pacc.tile([128, 256], F32, tag="acc")
            k = 0
            for cc in range(2):
                for s, (i, j) in enumerate(shifts):
                    lhsT = qb[:, cc, s, :]
                    rhs = fdp[cc][:, i + 8 * g: i + 8 * g + 8, j:j + 32]
                    nc.tensor.matmul(ps[:], lhsT, rhs,
                                     start=(k == 0), stop=(k == 17))
                    k += 1
            pss.append((g, ps))
        return pss

    def gram_stage(t):
        qb = qbp.tile([128, 2, 9, 128], gdt, tag="qb", name=f"qb_{rep}_{t}")
        for cc in range(2):
            for s, (i, j) in enumerate(shifts):
                nc.vector.tensor_copy(
                    qb[:, cc, s, :].rearrange("p (a b) -> p a b", a=4),
                    fdp[cc][:, i + 4 * t: i + 4 * t + 4, j:j + 32])
        return gram_matmuls(t, qb)

    def gram_finish(t, pss):
        for g, ps in pss:
            nc.vector.tensor_scalar_mul(
                M0[:, t, 1 + 256 * g: 1 + 256 * (g + 1)].bitcast(F32R),
                ps[:], rnorm_q[:, t:t + 1])
        # mirror lower blocks of row t: (t, g) for g < t//2 from rows 2g/2g+1
        for g in range(t // 2):
            psT = ptp.tile([128, 256], F32, tag="tp", name=f"mir_{rep}_{t}_{g}")
            for a in range(2):
                nc.tensor.transpose(
                    psT[:, 128 * a:128 * (a + 1)].bitcast(F32R),
                    M0[:, 2 * g + a, 1 + 128 * t: 1 + 128 * t + 128]
                    .bitcast(F32R),
                    identr)
            nc.vector.scalar_tensor_tensor(
                out=M0[:, t, 1 + 256 * g: 1 + 256 * (g + 1)].bitcast(F32R),
                in0=psT[:], scalar=rnorm_q[:, t:t + 1],
                in1=nrm_rep[:, 256 * g: 256 * (g + 1)],
                op0=OP.mult, op1=OP.mult)

    # t=0 matmuls run on PE while the DVE norm chain drains; the rnorm
    # transposes come after them so the in-order PE queue is never blocked
    pss0 = gram_stage(0)
    for c8 in range(8):
        nc.tensor.transpose(psq[:, 8 + c8:9 + c8],
                            rnm[0:1, 128 * c8:128 * (c8 + 1)], ident[0:1, 0:1])
    nc.vector.tensor_copy(rnorm_q[:], psq[:, 8:16])
    gram_finish(0, pss0)
    for h in range(2):
        psn = pacc.tile([128, 512], F32, tag="acc", name=f"nr_{rep}_{h}")
        nc.tensor.matmul(psn[:], ones1[:], nrm[0:1, 512 * h:512 * (h + 1)],
                         start=True, stop=True)
        nc.vector.tensor_copy(nrm_rep[:, 512 * h:512 * (h + 1)], psn[:])
    for t in range(1, 8):
        pss = gram_stage(t)
        gram_finish(t, pss)
        if t == 1:
            rw_group(0, 2)
        elif t == 3:
            rw_group(0, 3)
        elif t == 2:
            # mask column transport (mask chain surely drained by now)
            for c8 in range(8):
                nc.tensor.transpose(psq[:, c8:c8 + 1],
                                    mmrow[0:1, 128 * c8:128 * (c8 + 1)],
                                    ident[0:1, 0:1])
            nc.vector.tensor_copy(mm_q[:], psq[:, 0:8])

    for grp in range(4):
        rw_group(1, grp)

    # ---------------- stage 3: fuse1 (diag +-1, row-major) on PE ----------
    # M1[q, j] = M0[q, j] + M0[q+1, j+1] + M0[q-1, j-1] (integer q/p index,
    # zero at bounds).  Partition shifts by identity-slice matmuls; the
    # cross-chunk single rows via gpsimd accumulate-DMAs afterwards.
    M1 = wk.tile([128, 8, 1026], F32, tag="wk", name=f"m1_{rep}")
    nc.vector.memset(M1[:, :, 0:1026:1025], 0.0)
    for ch in range(8):
        for h in range(2):
            lo = 512 * h
            ps = pacc.tile([128, 512], F32, tag="acc")
            # center (padded data col = p + 1)
            nc.tensor.matmul(ps[:], identr,
                             M0[:, ch, 1 + lo:1 + lo + 512].bitcast(F32R),
                             start=True, stop=False)
            # +1 term: out[q, p] += M0[q+1, p+1] (p=1023 hits the zero pad)
            nc.tensor.matmul(ps[:], sh1p,
                             M0[:, ch, 2 + lo:2 + lo + 512].bitcast(F32R),
                             start=False, stop=False)
            # -1 term: out[q, p] += M0[q-1, p-1] (p=0 hits the zero pad)
            nc.tensor.matmul(ps[:], sh1m,
                             M0[:, ch, lo:lo + 512].bitcast(F32R),
                             start=False, stop=False)
            # cross-chunk stitch rows via single-element selector matmuls
            if ch < 7:
                nc.tensor.matmul(ps[:], st_p,
                                 M0[:, ch + 1, 2 + lo:2 + lo + 512]
                                 .bitcast(F32R),
                                 start=False, stop=(ch == 0))
            if ch > 0:
                nc.tensor.matmul(ps[:], st_m,
                                 M0[:, ch - 1, lo:lo + 512].bitcast(F32R),
                                 start=False, stop=True)
            if h == 0:
                nc.vector.tensor_copy(
                    M1[:, ch, 1 + lo:1 + lo + 512].bitcast(F32R), ps[:])
            else:
                nc.scalar.copy(M1[:, ch, 1 + lo:1 + lo + 512].bitcast(F32R),
                               ps[:])

    # ---------------- stage 4: fuse2 (diag +-1, col-major) on PE ----------
    # col-major +1 on integer index i (grid (a, b), i = b*32 + a):
    #   q side: q+32 for qy<=30; (qy=31, qx) -> qx+1 (chunk-7 wrap)
    #   p side: p+32 for py<=30; (py=31, px) -> px+1 (free-dim sliver)
    # mask mm_q folded into the PSUM->SBUF copies.
    M2 = wk.tile([128, 8, 1024], F32, tag="wk", name=f"m2_{rep}")

    def m1p(ch_, c0, c1):
        # padded column indexing: data col p lives at padded col p + 1
        return M1[:, ch_, c0:c1].bitcast(F32R)

    for ch in range(8):
        # q-side lhsT for the +1 / -1 col-major terms:
        #   main piece within chunk, cross piece from the adjacent chunk
        qp = [(sh32p, ch), (sh96m, ch + 1) if ch < 7 else (w7, 0)]
        qm = [(sh32m, ch), (sh96p, ch - 1) if ch > 0 else (w0, 7)]
        for h in range(2):
            lo = 512 * h
            ps = pacc.tile([128, 512], F32, tag="acc")
            mm = [(ps[:], identr, m1p(ch, 1 + lo, 1 + lo + 512))]
            sl = []   # sliver matmuls -> aligned scratch psum
            if h == 0:
                # +1: out cols 0..511 (py 0..15) <- src data 32..543
                for l, c in qp:
                    mm.append((ps[:], l, m1p(c, 33, 545)))
                # -1: out cols 32..511 (py 1..15) <- src data 0..479
                for l, c in qm:
                    mm.append((ps[:, 32:512], l, m1p(c, 1, 481)))
                # -1 sliver: out p 1..31 (py=0, px 1..31) <- data 991+p
                for l, c in qm:
                    sl.append((l, m1p(c, 992, 1024)))
                sadd = (1, 32, 1, 32)   # pssl[1:32] -> ps[1:32]
            else:
                # +1 main: out cols 0..479 (py 16..30) <- src data 544..1023
                for l, c in qp:
                    mm.append((ps[:, 0:480], l, m1p(c, 545, 1025)))
                # +1 sliver: out p 992..1022 (py=31, px 0..30) <- data px+1
                for l, c in qp:
                    sl.append((l, m1p(c, 2, 34)))
                sadd = (0, 31, 480, 511)  # pssl[0:31] -> ps[480:511]
                # -1: out cols 512..1023 (py 16..31) <- src data 480..991
                for l, c in qm:
                    mm.append((ps[:], l, m1p(c, 481, 993)))
            for k, (o, l, r) in enumerate(mm):
                nc.tensor.matmul(o, l, r, start=(k == 0),
                                 stop=(k == len(mm) - 1))
            pssl = ptp.tile([128, 32], F32, tag="tp", name=f"sl_{rep}_{ch}_{h}")
            for k, (l, r) in enumerate(sl):
                nc.tensor.matmul(pssl[:], l, r, start=(k == 0),
                                 stop=(k == len(sl) - 1))
            s0, s1, d0, d1 = sadd
            slv = wr.tile([128, 32], F32, tag="w", name=f"slv_{rep}_{ch}_{h}")
            nc.vector.tensor_copy(slv[:, s0:s1], pssl[:, s0:s1])
            nc.vector.tensor_add(ps[:, d0:d1], ps[:, d0:d1], slv[:, s0:s1])
            # masked copy out (fold mm_q)
            if h == 0:
                nc.vector.tensor_scalar_mul(
                    M2[:, ch, lo:lo + 512].bitcast(F32R), ps[:],
                    mm_q[:, ch:ch + 1])
            else:
                nc.scalar.mul(M2[:, ch, lo:lo + 512].bitcast(F32R), ps[:],
                              mm_q[:, ch:ch + 1])

    # ---------------- stage 5: max, subtract, exp ----------------
    mx8 = sml.tile([128, 8, 2], F32, tag="mx8")
    for pt in range(8):
        for g in range(2):
            ps = ptp.tile([128, 512], F32, tag="tp", name=f"tpb_{rep}_{pt}_{g}")
            for t4 in range(4):
                t = 4 * g + t4
                nc.tensor.transpose(
                    ps[:, 128 * t4:128 * (t4 + 1)].bitcast(F32R),
                    M2[:, t, 128 * pt:128 * (pt + 1)].bitcast(F32R),
                    identr)
            nc.vector.reduce_max(mx8[:, pt, g:g + 1], ps[:], axis=AX.X)
    mx_all = sml.tile([128, 8], F32, tag="mx_all")
    for pt in range(8):
        nc.vector.reduce_max(mx_all[:, pt:pt + 1], mx8[:, pt, :], axis=AX.X)
    # max col [128, 8] -> row [1, 1024] via PE transposes
    mxrow = sml.tile([1, 1024], F32, tag="mxrow")
    for g in range(2):
        psr = ptp.tile([1, 512], F32, tag="tp", name=f"mxr_{rep}_{g}")
        for c4 in range(4):
            pt = 4 * g + c4
            nc.tensor.transpose(psr[0:1, 128 * c4:128 * (c4 + 1)],
                                mx_all[:, pt:pt + 1], ident[:])
        nc.vector.tensor_copy(mxrow[:, 512 * g:512 * (g + 1)], psr[:])
    E = wk.tile([128, 8, 1024], BF16, tag="wk", name=f"e_{rep}")
    psrs = []
    bcs = qbp.tile([128, 1024], F32, tag="qb", name=f"bcs_{rep}")
    for h in range(2):
        psr = pacc.tile([128, 512], F32, tag="acc", name=f"bc_{rep}_{h}")
        nc.tensor.matmul(psr[:], ones1[:], mxrow[:, 512 * h:512 * (h + 1)],
                         start=True, stop=True)
        nc.scalar.copy(bcs[:, 512 * h:512 * (h + 1)], psr[:])
        psrs.append(psr)
    # subtract/exp/denominator pipelined at (t, h) granularity; both
    # denominator PSUM groups stay open while exps stream in
    rcp = sml.tile([128, 1024], F32, tag="rcp")
    pssd = [pacc.tile([128, 512], F32, tag="acc", name=f"dn_{rep}_{h}")
            for h in range(2)]
    for t in range(8):
        for h in range(2):
            st = wr.tile([128, 512], F32, tag="w", name=f"st_{rep}_{t}_{h}")
            if h == 0:
                nc.vector.tensor_tensor(
                    st[:], M2[:, t, 512 * h:512 * (h + 1)], psrs[h][:],
                    OP.subtract)
            else:
                nc.gpsimd.tensor_tensor(
                    st[:], M2[:, t, 512 * h:512 * (h + 1)],
                    bcs[:, 512 * h:512 * (h + 1)], OP.subtract)
            nc.scalar.activation(E[:, t, 512 * h:512 * (h + 1)], st[:],
                                 ACT.Exp, bias=0.0, scale=SCALE)
            nc.tensor.matmul(pssd[h][:], ones_bf[:],
                             E[:, t, 512 * h:512 * (h + 1)],
                             start=(t == 0), stop=(t == 7))
    for h in range(2):
        nc.vector.reciprocal(rcp[:, 512 * h:512 * (h + 1)], pssd[h][:])

    # ---------------- stage 5c: final weights -> A_pad ----------------
    A_pad = wk.tile([128, 8, 34, 34], ddt, tag="wk", name=f"ap_{rep}")
    nc.gpsimd.memset(A_pad[:, :, 0:34:33, :], 0.0)
    nc.gpsimd.memset(A_pad[:, :, 1:33, 0:34:33], 0.0)
    # weight build split into row halves so the top-half deconv can start
    # while bottom-half weights are still being produced
    for (r0, r1) in ((1, 19), (19, 33)):
        for t in range(8):
            nc.vector.scalar_tensor_tensor(
                out=A_pad[:, t, r0:r1, 1:33],
                in0=E[:, t, :].rearrange("p (a b) -> p a b", a=32)
                [:, r0 - 1:r1 - 1, :],
                scalar=mm_q[:, t:t + 1],
                in1=rcp[:].rearrange("p (a b) -> p a b", a=32)
                [:, r0 - 1:r1 - 1, :],
                op0=OP.mult, op1=OP.mult)

    # ---------------- stage 6: deconv ----------------
    for cc in range(2):
        out_sb = wk.tile([128, 64, 64], F32, tag="wk", name=f"os_{rep}_{cc}")
        od = out_d[cc * 128:(cc + 1) * 128]
        for h in range(2):
            accs, cnt = {}, {}
            for ry in range(2):
                for rx in range(2):
                    accs[(ry, rx)] = pacc.tile(
                        [128, 512], F32, tag="acc",
                        name=f"da_{rep}_{cc}_{h}_{ry}_{rx}")
                    cnt[(ry, rx)] = 0
            for qc in range(8):
                for ry in range(2):
                    us = [u for u in range(4) if (u + 1) % 2 == ry]
                    for rx in range(2):
                        vs = [v for v in range(4) if (v + 1) % 2 == rx]
                        for u in us:
                            for v in vs:
                                sy = (ry + 1 - u) // 2
                                sx = (rx + 1 - v) // 2
                                rhs = A_pad[:, qc,
                                            1 + sy + 16 * h: 1 + sy + 16 * h + 16,
                                            1 + sx: 1 + sx + 32]
                                k = cnt[(ry, rx)]
                                nc.tensor.matmul(accs[(ry, rx)][:],
                                                 RW[cc][:, qc, 4 * u + v, :], rhs,
                                                 start=(k == 0), stop=(k == 31))
                                cnt[(ry, rx)] += 1
            for ry in range(2):
                for rx in range(2):
                    dst = out_sb[:, 32 * h + ry: 32 * (h + 1): 2, rx::2]
                    if rx == 0:
                        nc.vector.tensor_scalar_mul(dst, accs[(ry, rx)][:], 0.25)
                    else:
                        nc.scalar.mul(dst, accs[(ry, rx)][:], 0.25)
            eng = nc.sync if (cc + h) % 2 == 0 else nc.scalar
            eng.dma_start(od[:, 32 * h:32 * h + 32],
                          out_sb[:, 32 * h:32 * h + 32])


_NC_CACHE = {}


def _get_nc(cfg=("f32r", "bf16")):
    if cfg not in _NC_CACHE:
        _NC_CACHE[cfg] = build_nc(*cfg)
    return _NC_CACHE[cfg]


def kernel(feature: np.ndarray, mask: np.ndarray) -> np.ndarray:
    feature = np.ascontiguousarray(np.asarray(feature, dtype=np.float32))
    mask = np.asarray(mask, dtype=np.float32)
    nc = _get_nc()
    m0 = np.ascontiguousarray(mask[0, 0])
    in_maps = [{"feature": np.ascontiguousarray(feature[i]), "mask0": m0}
               for i in range(N_CORES)]
    res = run_bass_kernel_spmd(nc, in_maps, list(range(N_CORES)))
    return np.stack([np.asarray(res.results[i]["out"], dtype=np.float32)
                     for i in range(N_CORES)])
